# revision 4
# baseline (speedup 1.0000x reference)
"""Causal self-attention (B=4, T=2048, C=1024, H=16) on 8 Trainium2 cores.

Sharding: core i handles batch b = i//2 and head-group g = i%2 (8 heads,
512 channels). Host sums the two head-group partials per batch and adds
the bias row (v-bias folded through W_proj; k-bias cancels in softmax;
q-bias added on-device).

Design (~231us/core, vs 332us for the f32r baseline):
- x and the weights arrive bf16; xT is produced by XBAR DMA-transpose
  straight from DRAM (no PE transposes, no DVE copy-backs). All PE
  matmuls run at 1 cycle/row.
- AV matmul in natural layout: out[q 128, 65] per (head, qblock) with a
  fused ones-column producing the softmax denominator as column 64 ->
  F=65 moving rows instead of the yT layout's F=512 per key block
  (halves PE time on AV). Normalization is one reciprocal + one
  free-broadcast multiply per head per chunk.
- y transposed back to chan-major via XBAR DMA-transpose per query
  block; the last chunk's final head-pair is transposed on the (then
  idle) PE instead, shortening the tail.
- Diagonal score pairs compute only valid column windows, packed
  adjacently so one exp + one mask-mul covers a whole pair. The small
  diagonal pair gets its own 1-bank PSUM pool; AV accumulators share
  the matmul-drain pool (frees a PSUM bank for score-tile rotation).
- QKV (next quarter), attention (current chunk), and projection (prev
  chunks, concentrated where ACT exp pressure peaks) are emitted
  interleaved; all DMAs go through the SP queue (the tile scheduler
  pins cross-queue DMA order with completion semaphores).
"""

import sys
from collections import deque

import numpy as np

sys.path.insert(0, "/opt/trn_rl_repo")

import concourse.bass as bass  # noqa: E402
import concourse.mybir as mybir  # noqa: E402
from concourse.tile import TileContext  # noqa: E402

F32 = mybir.dt.float32
BF16 = mybir.dt.bfloat16
EXP = mybir.ActivationFunctionType.Exp
IS_GE = mybir.AluOpType.is_ge

B, T, C, H, D = 4, 2048, 1024, 16, 64
NCORES = 8
HL = 8          # heads per core
CL = HL * D     # 512 local channels
KC = C // 128   # 8 contraction chunks
TCH = T // 128  # 16 key blocks of 128
NT = T // 512   # 4 query chunks of 512
SCALE = 1.0 / 8.0  # 1/sqrt(64)


# --------------------------------------------------------------------------
# Workaround: this walrus build accepts only ONE sync-wait per instruction.
# Split extras onto fresh single-wait EventSemaphore instructions.
# --------------------------------------------------------------------------
def _split_multiwait_insts(nc):
    ctr = 0
    for f in nc.m.functions:
        for blk in f.blocks:
            insts = list(blk.instructions)
            new_list = []
            changed = False
            for inst in insts:
                si = inst.sync_info
                if si is not None and len(si.on_wait) > 1:
                    waits = list(si.on_wait)
                    keep_idx = len(waits) - 1
                    for i, w in enumerate(waits):
                        if w.wait_reg is not None:
                            keep_idx = i
                            break
                    for i, w in enumerate(waits):
                        if i == keep_idx:
                            continue
                        ev = mybir.InstEventSemaphore(
                            name=f"evsplit_{ctr}", ins=[], outs=[]
                        )
                        ctr += 1
                        ev.engine = inst.engine
                        ev.sync_info = mybir.SyncInfo(on_wait=[w], on_update=[])
                        new_list.append(ev)
                    inst.sync_info.on_wait = [waits[keep_idx]]
                    changed = True
                new_list.append(inst)
            if changed:
                blk.instructions = new_list


def build_bass(repeat=1):
    nc = bass.Bass("TRN2", target_bir_lowering=False, debug=False)

    xb_d = nc.dram_tensor("xb", [T, C], BF16, kind="ExternalInput")
    wqk_d = nc.dram_tensor("wqk", [C, 2 * CL], BF16, kind="ExternalInput")
    wv_d = nc.dram_tensor("wv", [C, CL], BF16, kind="ExternalInput")
    bq_d = nc.dram_tensor("bq", [4, 128, 1], F32, kind="ExternalInput")
    wp_d = nc.dram_tensor("wp", [CL, C], BF16, kind="ExternalInput")
    out_d = nc.dram_tensor("out", [T, C], F32, kind="ExternalOutput")

    with TileContext(nc) as tc:
        for _rep in range(repeat):
            _emit_body(nc, tc, xb_d, wqk_d, wv_d, bq_d, wp_d, out_d)

    _split_multiwait_insts(nc)
    return nc


def _emit_body(nc, tc, xb_d, wqk_d, wv_d, bq_d, wp_d, out_d):
    from contextlib import ExitStack

    with ExitStack() as ctx:
        const = ctx.enter_context(tc.tile_pool(name="const", bufs=1))
        big = ctx.enter_context(tc.tile_pool(name="big", bufs=1))
        xtp = ctx.enter_context(tc.tile_pool(name="xtp", bufs=2))
        e_pool = ctx.enter_context(tc.tile_pool(name="epool", bufs=22))
        ynp = ctx.enter_context(tc.tile_pool(name="ynp", bufs=2))
        rcp = ctx.enter_context(tc.tile_pool(name="rcp", bufs=3))
        osp = ctx.enter_context(tc.tile_pool(name="osp", bufs=3))
        mm_ps = ctx.enter_context(tc.tile_pool(name="mmps", bufs=3, space="PSUM"))
        qk_ps = ctx.enter_context(tc.tile_pool(name="qkps", bufs=2, space="PSUM"))
        qkd_ps = ctx.enter_context(tc.tile_pool(name="qkdps", bufs=1, space="PSUM"))

        # ---- persistent tensors ----
        qkT = big.tile([128, 8, T], BF16)          # m 0-3: q chan blocks, 4-7: k
        vaug = big.tile([128, TCH, HL, 65], BF16)  # v natural + ones col (64)
        yT = big.tile([128, 4, T], BF16)           # chan-major y for c_proj
        wqk_sb = big.tile([128, KC, 2 * CL], BF16)
        wv_sb = big.tile([128, KC, CL], BF16)
        wp_sb = big.tile([128, 4, C], BF16)
        bq_sb = const.tile([128, 4, 1], F32)
        idn = const.tile([128, 128], BF16)
        # Packed causal masks for the two diagonal score pairs: the valid
        # column windows of both halves are packed adjacently, so one exp +
        # one mask-mul covers a whole pair.
        m896 = const.tile([128, 896], BF16)   # [tri512 | tri384]
        m384 = const.tile([128, 384], BF16)   # [tri256 | tri128]

        # ---- startup DMAs (order matters: PE's first work is v then qk) ----
        xTs = [None] * NT

        def emit_xt(c, split=False):
            xt = xtp.tile([128, KC, 512], BF16, tag="xT", name=f"xT{c}")
            if split:  # halves so the first v-block unblocks sooner
                for hh in range(2):
                    nc.sync.dma_start_transpose(
                        out=xt[:, :, hh * 256:(hh + 1) * 256],
                        in_=xb_d.ap()[c * 512 + hh * 256:
                                      c * 512 + (hh + 1) * 256, :],
                    )
            else:
                nc.sync.dma_start_transpose(
                    out=xt, in_=xb_d.ap()[c * 512:(c + 1) * 512, :]
                )
            xTs[c] = xt

        # All DMAs go through the SP queue: the tile scheduler pins
        # cross-queue DMA order with completion semaphores (~2us dead time
        # per pin), while same-queue DMAs pipeline back-to-back.
        xt0 = xtp.tile([128, KC, 512], BF16, tag="xT", name="xT0")
        xTs[0] = xt0
        nc.sync.dma_start_transpose(
            out=xt0[:, :, 0:256], in_=xb_d.ap()[0:256, :]
        )
        nc.sync.dma_start(
            out=wv_sb[:, 0:4, :],
            in_=wv_d.ap()[0:512, :].rearrange("(k p) f -> p k f", p=128),
        )
        nc.sync.dma_start(
            out=wv_sb[:, 4:8, :],
            in_=wv_d.ap()[512:1024, :].rearrange("(k p) f -> p k f", p=128),
        )
        nc.sync.dma_start_transpose(
            out=xt0[:, :, 256:512], in_=xb_d.ap()[256:512, :]
        )
        nc.sync.dma_start(
            out=wqk_sb, in_=wqk_d.ap().rearrange("(k p) f -> p k f", p=128)
        )
        nc.sync.dma_start(out=bq_sb, in_=bq_d.ap().rearrange("a p o -> p a o"))
        # xT(1)/wp are issued later (inside QKV(0)) so their DMA requests
        # cannot jump ahead of wqk on the shared DMA engines.

        # masks / ones (DVE+Pool while DMAs stream)
        for mt, widths in ((m896, (512, 384)), (m384, (256, 128))):
            nc.vector.memset(mt, 1.0)
            off = 0
            for w in widths:
                nc.gpsimd.affine_select(
                    out=mt[:, off:off + w], in_=mt[:, off:off + w],
                    pattern=[[1, w]], compare_op=IS_GE, fill=0.0,
                    base=0, channel_multiplier=-1,
                )
                off += w
        nc.vector.memset(vaug[:, :, :, 64:65], 1.0)
        from concourse.masks import make_identity
        make_identity(nc, idn)

        # ---- work units ----
        # unit order: v blocks first (smaller weight tensor arrives first at
        # startup), then qk m-blocks interleaved q/k so that heads 2l/2l+1
        # (which read q block l and k block 4+l) unblock in head order.
        UNIT_ORDER = [("v", 0), ("v", 1), ("v", 2), ("v", 3),
                      ("m", 0), ("m", 4), ("m", 1), ("m", 5),
                      ("m", 2), ("m", 6), ("m", 3), ("m", 7)]

        def qkv_unit(c, u):
            kind, idx = UNIT_ORDER[u]
            if kind == "v":
                tt = idx
                pv = mm_ps.tile([128, CL], F32, tag="mm", name=f"pv{c}_{tt}")
                for k in range(KC):
                    nc.tensor.matmul(
                        out=pv,
                        lhsT=xTs[c][:, k, tt * 128:(tt + 1) * 128],
                        rhs=wv_sb[:, k, :],
                        start=(k == 0),
                        stop=(k == KC - 1),
                    )
                nc.vector.tensor_copy(
                    out=vaug[:, 4 * c + tt, :, 0:64],
                    in_=pv.rearrange("p (h d) -> p h d", d=64),
                )
            else:
                m = idx
                pq = mm_ps.tile([128, 512], F32, tag="mm", name=f"pq{c}_{m}")
                for k in range(KC):
                    nc.tensor.matmul(
                        out=pq,
                        lhsT=wqk_sb[:, k, m * 128:(m + 1) * 128],
                        rhs=xTs[c][:, k, :],
                        start=(k == 0),
                        stop=(k == KC - 1),
                    )
                dst = qkT[:, m, c * 512:(c + 1) * 512]
                if m < 4:  # q: add per-partition bias
                    nc.vector.tensor_scalar_add(dst, pq, bq_sb[:, m, :])
                else:
                    nc.vector.tensor_copy(out=dst, in_=pq)

        def scores_pair(c, l, pj, e_tiles):
            row = (l % 2) * 64
            qtile = l // 2
            ktile = 4 + l // 2
            j0 = 2 * pj - 4 * c
            los = {0: (0, 128), 2: (256, 384)}.get(j0, None)
            if j0 == 2:
                pqk = qkd_ps.tile([128, 384], F32, tag="qkd",
                                  name=f"pqk{c}_{l}_{pj}")
            else:
                pqk = qk_ps.tile([128, 1024], F32, tag="qk",
                                 name=f"pqk{c}_{l}_{pj}")
            e = e_pool.tile([128, 1024], BF16, tag="e", name=f"e{c}_{l}_{pj}")
            if los:
                # diagonal pairs: both halves' valid windows packed
                # adjacently -> one exp + one mask-mul for the pair
                lo0, lo1 = los
                w0, w1 = 512 - lo0, 512 - lo1
                for h, (lo, base) in enumerate(((lo0, 0), (lo1, w0))):
                    nc.tensor.matmul(
                        out=pqk[:, base:base + 512 - lo],
                        lhsT=qkT[row:row + 64, ktile,
                                 (2 * pj + h) * 128:(2 * pj + h + 1) * 128],
                        rhs=qkT[row:row + 64, qtile,
                                c * 512 + lo:(c + 1) * 512],
                        start=True,
                        stop=True,
                    )
                w = w0 + w1
                nc.scalar.activation(
                    out=e[:, 0:w], in_=pqk[:, 0:w], func=EXP, scale=SCALE,
                )
                nc.vector.tensor_mul(
                    e[:, 0:w], e[:, 0:w], m896 if w == 896 else m384
                )
            else:
                for h in range(2):
                    tk = 2 * pj + h
                    nc.tensor.matmul(
                        out=pqk[:, h * 512:(h + 1) * 512],
                        lhsT=qkT[row:row + 64, ktile,
                                 tk * 128:(tk + 1) * 128],
                        rhs=qkT[row:row + 64, qtile,
                                c * 512:(c + 1) * 512],
                        start=True,
                        stop=True,
                    )
                nc.scalar.activation(out=e, in_=pqk, func=EXP, scale=SCALE)
            e_tiles.append(e)

        yv_tiles = {}

        def av_chain(c, l, qq, e_tiles):
            if l not in yv_tiles:
                yvt = mm_ps.tile([128, 512], F32, tag="mm", name=f"yv{c}_{l}")
                yv_tiles[l] = yvt[:, 0:260].rearrange("p (q e) -> p q e", e=65)
            yv = yv_tiles[l]
            nk = 4 * c + qq + 1
            for tk in range(nk):
                pj, h = tk // 2, tk % 2
                j0 = 2 * pj - 4 * c
                los = {0: (0, 128), 2: (256, 384)}.get(j0, None)
                if los:  # packed diagonal-pair e layout
                    lo0, lo1 = los
                    col = (qq * 128 - lo0) if h == 0 else \
                        (512 - lo0) + (qq * 128 - lo1)
                else:
                    col = h * 512 + qq * 128
                nc.tensor.matmul(
                    out=yv[:, qq, :],
                    lhsT=e_tiles[pj][:, col:col + 128],
                    rhs=vaug[:, tk, l, :],
                    start=(tk == 0),
                    stop=(tk == nk - 1),
                )

        def norm_head(c, l, ynat):
            yv = yv_tiles.pop(l)
            rc = rcp.tile([128, 4], F32, tag="rc", name=f"rc{c}_{l}")
            nc.vector.reciprocal(out=rc, in_=yv[:, :, 64])
            nc.vector.tensor_mul(
                ynat[:, :, l * 64:(l + 1) * 64],
                yv[:, :, 0:64],
                rc.unsqueeze(2).broadcast_to((128, 4, 64)),
            )

        os_tiles = {}

        def proj_unit(c, u):
            tq = 4 * c + u // 2
            oc = u % 2
            if oc == 0:
                os_tiles[tq] = osp.tile([128, 1024], F32, tag="os",
                                        name=f"os{tq}")
            os_ = os_tiles[tq]
            pp = mm_ps.tile([128, 512], F32, tag="mm", name=f"pp{tq}_{oc}")
            for k in range(4):
                nc.tensor.matmul(
                    out=pp,
                    lhsT=yT[:, k, tq * 128:(tq + 1) * 128],
                    rhs=wp_sb[:, k, oc * 512:(oc + 1) * 512],
                    start=(k == 0),
                    stop=(k == 3),
                )
            nc.vector.tensor_copy(out=os_[:, oc * 512:(oc + 1) * 512], in_=pp)
            nc.sync.dma_start(
                out=out_d.ap()[tq * 128:(tq + 1) * 128,
                               oc * 512:(oc + 1) * 512],
                in_=os_[:, oc * 512:(oc + 1) * 512],
            )
            if oc == 1:
                del os_tiles[tq]

        # ---- main pipeline ----
        # QKV(0) first 8 units straight (PE's first work; heads 0-3 of
        # chunk 0 unblock), rest deferred into chunk 0's filler stream.
        # Per chunk c: the scores of head l+1 are interleaved at pair
        # granularity with head l's AV chains and with QKV/proj filler
        # units, so PE never sits in an ACT-paced scores run. proj(1) and
        # proj(2) are both deferred to chunk 3, where ACT exp pressure
        # peaks and PE needs the most filler work.
        for u in range(8):
            qkv_unit(0, u)
            if u == 4:
                emit_xt(1)
            if u == 6:
                nc.sync.dma_start(
                    out=wp_sb,
                    in_=wp_d.ap().rearrange("(k p) f -> p k f", p=128),
                )

        for c in range(NT):
            npairs = 2 * c + 2
            if c + 2 < NT:
                emit_xt(c + 2)
            fillers = deque()
            if c == 0:
                for u in range(8, 12):
                    fillers.append(lambda u=u: qkv_unit(0, u))
            if c + 1 < NT:
                for u in range(12):
                    fillers.append(lambda u=u: qkv_unit(c + 1, u))
            if c == 1:
                for u in range(8):
                    fillers.append(lambda u=u: proj_unit(0, u))
            if c == 3:
                for cc in (1, 2):
                    for u in range(8):
                        fillers.append(lambda cc=cc, u=u: proj_unit(cc, u))
            nf = len(fillers)
            total_slots = HL * npairs
            done = [0]

            def tick(slot, nf=nf, total_slots=total_slots, done=done,
                     fillers=fillers):
                want = (nf * slot) // total_slots
                while done[0] < want and fillers:
                    fillers.popleft()()
                    done[0] += 1

            ynat = ynp.tile([128, 4, 512], BF16, tag="ynat", name=f"ynat{c}")
            e_heads = [[] for _ in range(HL)]
            for pj in range(npairs):
                scores_pair(c, 0, pj, e_heads[0])
            for l in range(HL):
                # spread head l's 4 AV chains across head l+1's score pairs
                if l + 1 < HL:
                    av_at = {((qq + 1) * npairs) // 5: qq for qq in range(4)}
                    for pj in range(npairs):
                        scores_pair(c, l + 1, pj, e_heads[l + 1])
                        tick(l * npairs + pj + 1)
                        if pj in av_at:
                            av_chain(c, l, av_at[pj], e_heads[l])
                    for qq in range(4):  # c=0 has only 2 pair slots
                        if qq not in av_at.values():
                            av_chain(c, l, qq, e_heads[l])
                else:
                    for qq in range(4):
                        av_chain(c, l, qq, e_heads[l])
                        tick(l * npairs + ((qq + 1) * npairs) // 4)
                norm_head(c, l, ynat)
                if c == 3 and l % 2 == 1 and l < 7:
                    p = l // 2
                    for qq in range(4):
                        nc.sync.dma_start_transpose(
                            out=yT[:, p,
                                   (4 * c + qq) * 128:(4 * c + qq + 1) * 128],
                            in_=ynat[:, qq, p * 128:(p + 1) * 128],
                        )
                if c == 3 and l == 7:
                    # last pair: PE transposes (PE is idle here; skips the
                    # serial DMA issue latency on the critical tail path)
                    tp = qk_ps.tile([128, 1024], F32, tag="qk",
                                    name="tps3").bitcast(BF16)
                    for qq in range(4):
                        nc.tensor.transpose(
                            out=tp[:, qq * 128:(qq + 1) * 128],
                            in_=ynat[:, qq, 384:512],
                            identity=idn,
                        )
                    for qq in range(4):
                        nc.vector.tensor_copy(
                            out=yT[:, 3, (12 + qq) * 128:(13 + qq) * 128],
                            in_=tp[:, qq * 128:(qq + 1) * 128],
                        )
            while fillers:
                fillers.popleft()()
            if c < 3:
                for qq in range(4):
                    nc.sync.dma_start_transpose(
                        out=yT[:, :, (4 * c + qq) * 128:(4 * c + qq + 1) * 128],
                        in_=ynat[:, qq, :],
                    )
        for u in range(8):
            proj_unit(3, u)


# --------------------------------------------------------------------------
# Cached PJRT execution (reuses the compiled executable across calls).
# --------------------------------------------------------------------------
_CACHE = {}


def _get_runner(repeat=1):
    key = ("runner", repeat)
    if key in _CACHE:
        return _CACHE[key]

    import jax
    from jax.sharding import Mesh, PartitionSpec
    from jax.experimental.shard_map import shard_map
    from concourse import bass2jax

    nc = build_bass(repeat=repeat)
    bass2jax.install_neuronx_cc_hook()

    partition_name = (
        nc.partition_id_tensor.name if nc.partition_id_tensor else None
    )
    in_names, out_names, out_avals, zero_shapes = [], [], [], []
    for alloc in nc.m.functions[0].allocations:
        if not isinstance(alloc, mybir.MemoryLocationSet):
            continue
        name = alloc.memorylocations[0].name
        if alloc.kind == "ExternalInput":
            if name != partition_name:
                in_names.append(name)
        elif alloc.kind == "ExternalOutput":
            shape = tuple(alloc.tensor_shape)
            dtype = mybir.dt.np(alloc.dtype)
            out_names.append(name)
            out_avals.append(jax.core.ShapedArray(shape, dtype))
            zero_shapes.append((shape, dtype))
    n_params = len(in_names)
    n_outs = len(out_avals)
    all_in_names = list(in_names) + list(out_names)
    if partition_name is not None:
        all_in_names.append(partition_name)

    def _body(*args):
        operands = list(args)
        if partition_name is not None:
            operands.append(bass2jax.partition_id_tensor())
        outs = bass2jax._bass_exec_p.bind(
            *operands,
            out_avals=tuple(out_avals),
            in_names=tuple(all_in_names),
            out_names=tuple(out_names),
            lowering_input_output_aliases=(),
            sim_require_finite=True,
            sim_require_nnan=True,
            nc=nc,
        )
        return tuple(outs)

    devices = jax.devices()[:NCORES]
    mesh = Mesh(np.asarray(devices), ("core",))
    in_specs = (PartitionSpec("core"),) * (n_params + n_outs)
    out_specs = (PartitionSpec("core"),) * n_outs
    donate = tuple(range(n_params, n_params + n_outs))
    sharded = jax.jit(
        shard_map(
            _body, mesh=mesh, in_specs=in_specs, out_specs=out_specs,
            check_rep=False,
        ),
        donate_argnums=donate,
        keep_unused=True,
    )

    runner = {
        "sharded": sharded,
        "in_names": in_names,
        "out_names": out_names,
        "zero_shapes": zero_shapes,
        "n_params": n_params,
        "mesh": mesh,
    }
    _CACHE[key] = runner
    return runner


def _make_core_inputs(x, W_attn, b_attn, W_proj):
    """Per-core input dicts (core i: batch i//2, head-group i%2)."""
    import ml_dtypes

    bf16 = ml_dtypes.bfloat16
    x = np.ascontiguousarray(x, dtype=np.float32)
    W_attn = np.ascontiguousarray(W_attn, dtype=np.float32)
    b_attn = np.ascontiguousarray(b_attn, dtype=np.float32)
    W_proj = np.ascontiguousarray(W_proj, dtype=np.float32)

    per_group = []
    for g in range(2):
        s = g * CL
        wqk = np.ascontiguousarray(
            np.concatenate(
                [W_attn[:, s:s + CL], W_attn[:, C + s:C + s + CL]], axis=1
            ).astype(bf16)
        )
        wv = np.ascontiguousarray(W_attn[:, 2 * C + s:2 * C + s + CL].astype(bf16))
        bq = np.ascontiguousarray(b_attn[s:s + CL].reshape(4, 128, 1))
        wp = np.ascontiguousarray(W_proj[s:s + CL, :].astype(bf16))
        per_group.append((wqk, wv, bq, wp))

    in_maps = []
    for core in range(NCORES):
        b_i, g = core // 2, core % 2
        wqk, wv, bq, wp = per_group[g]
        in_maps.append(
            {"xb": np.ascontiguousarray(x[b_i].astype(bf16)), "wqk": wqk,
             "wv": wv, "bq": bq, "wp": wp}
        )
    return in_maps


def run_cores(in_maps, timing_reps=0, repeat=1):
    """Run the SPMD kernel. Returns (list of per-core output dicts, best_ns)."""
    import jax, time

    r = _get_runner(repeat=repeat)
    per_core = [
        [np.asarray(m[name]) for name in r["in_names"]] for m in in_maps
    ]
    concat_in = [
        np.concatenate([per_core[c][i] for c in range(NCORES)], axis=0)
        for i in range(len(r["in_names"]))
    ]

    def zeros():
        return [
            np.zeros((NCORES * s[0], *s[1:]), dt) for (s, dt) in r["zero_shapes"]
        ]

    out_arrs = r["sharded"](*concat_in, *zeros())
    outs_np = [np.asarray(a) for a in out_arrs]

    best_ns = None
    if timing_reps > 0:
        from jax.sharding import NamedSharding, PartitionSpec

        shard = NamedSharding(r["mesh"], PartitionSpec("core"))
        dev_in = [jax.device_put(a, shard) for a in concat_in]
        for a in dev_in:
            a.block_until_ready()
        zsets = []
        for _ in range(timing_reps + 1):
            zs = [jax.device_put(z, shard) for z in zeros()]
            for a in zs:
                a.block_until_ready()
            zsets.append(zs)
        res = r["sharded"](*dev_in, *zsets[0])  # warm
        for a in res:
            a.block_until_ready()
        times = []
        for i in range(timing_reps):
            t0 = time.perf_counter()
            res = r["sharded"](*dev_in, *zsets[i + 1])
            for a in res:
                a.block_until_ready()
            t1 = time.perf_counter()
            times.append(t1 - t0)
        best_ns = int(min(times) * 1e9)

    results = []
    for c in range(NCORES):
        m = {}
        for i, name in enumerate(r["out_names"]):
            full = outs_np[i]
            shape = r["zero_shapes"][i][0]
            m[name] = full.reshape(NCORES, *shape)[c]
        results.append(m)
    return results, best_ns


def kernel(x, W_attn, b_attn, W_proj, b_proj, _timing_reps=0, _return_ns=False):
    x = np.asarray(x, dtype=np.float32)
    W_attn = np.asarray(W_attn, dtype=np.float32)
    b_attn = np.asarray(b_attn, dtype=np.float32)
    W_proj = np.asarray(W_proj, dtype=np.float32)
    b_proj = np.asarray(b_proj, dtype=np.float32)

    in_maps = _make_core_inputs(x, W_attn, b_attn, W_proj)
    results, best_ns = run_cores(in_maps, timing_reps=_timing_reps)

    # v-bias contributes a constant row through the projection
    bias_row = (b_proj + b_attn[2 * C:3 * C] @ W_proj).astype(np.float32)

    out = np.empty((B, T, C), dtype=np.float32)
    for b_i in range(B):
        out[b_i] = results[2 * b_i]["out"] + results[2 * b_i + 1]["out"]
        out[b_i] += bias_row[None, :]
    if _return_ns:
        return out, best_ns
    return out


# revision 5
# speedup vs baseline: 1.0268x; 1.0268x over previous
"""Causal self-attention (B=4, T=2048, C=1024, H=16) on 8 Trainium2 cores.

Sharding: core i handles batch b = i//2 and head-group g = i%2 (8 heads,
512 channels). Host sums the two head-group partials per batch and adds
the bias row (v-bias folded through W_proj; k-bias cancels in softmax;
q-bias added on-device).

Design (~225us/core, vs 332us for the f32r baseline):
- x and the weights arrive bf16; xT is produced by XBAR DMA-transpose
  straight from DRAM (no PE transposes, no DVE copy-backs). All PE
  matmuls run at 1 cycle/row.
- AV matmul in natural layout: out[q 128, 65] per (head, qblock) with a
  fused ones-column producing the softmax denominator as column 64 ->
  F=65 moving rows instead of the yT layout's F=512 per key block
  (halves PE time on AV). Normalization is one reciprocal + one
  free-broadcast multiply per head per chunk.
- y transposed back to chan-major via XBAR DMA-transpose per query
  block; the last chunk's final head-pair is transposed on the (then
  idle) PE instead, shortening the tail.
- Diagonal score pairs compute only valid column windows, packed
  adjacently so one exp + one mask-mul covers a whole pair. The small
  diagonal pair gets its own 1-bank PSUM pool; AV accumulators share
  the matmul-drain pool (frees a PSUM bank for score-tile rotation).
- All c_proj work is deferred into the last attention chunk, where ACT
  exp pressure peaks and PE needs filler work; all DMAs go through the
  SP queue (the tile scheduler pins cross-queue DMA order with
  completion semaphores).
"""

import sys
from collections import deque

import numpy as np

sys.path.insert(0, "/opt/trn_rl_repo")

import concourse.bass as bass  # noqa: E402
import concourse.mybir as mybir  # noqa: E402
from concourse.tile import TileContext  # noqa: E402

F32 = mybir.dt.float32
BF16 = mybir.dt.bfloat16
EXP = mybir.ActivationFunctionType.Exp
IS_GE = mybir.AluOpType.is_ge

B, T, C, H, D = 4, 2048, 1024, 16, 64
NCORES = 8
HL = 8          # heads per core
CL = HL * D     # 512 local channels
KC = C // 128   # 8 contraction chunks
TCH = T // 128  # 16 key blocks of 128
NT = T // 512   # 4 query chunks of 512
SCALE = 1.0 / 8.0  # 1/sqrt(64)


# --------------------------------------------------------------------------
# Workaround: this walrus build accepts only ONE sync-wait per instruction.
# Split extras onto fresh single-wait EventSemaphore instructions.
# --------------------------------------------------------------------------
def _split_multiwait_insts(nc):
    ctr = 0
    for f in nc.m.functions:
        for blk in f.blocks:
            insts = list(blk.instructions)
            new_list = []
            changed = False
            for inst in insts:
                si = inst.sync_info
                if si is not None and len(si.on_wait) > 1:
                    waits = list(si.on_wait)
                    keep_idx = len(waits) - 1
                    for i, w in enumerate(waits):
                        if w.wait_reg is not None:
                            keep_idx = i
                            break
                    for i, w in enumerate(waits):
                        if i == keep_idx:
                            continue
                        ev = mybir.InstEventSemaphore(
                            name=f"evsplit_{ctr}", ins=[], outs=[]
                        )
                        ctr += 1
                        ev.engine = inst.engine
                        ev.sync_info = mybir.SyncInfo(on_wait=[w], on_update=[])
                        new_list.append(ev)
                    inst.sync_info.on_wait = [waits[keep_idx]]
                    changed = True
                new_list.append(inst)
            if changed:
                blk.instructions = new_list


def build_bass(repeat=1):
    nc = bass.Bass("TRN2", target_bir_lowering=False, debug=False)

    xb_d = nc.dram_tensor("xb", [T, C], BF16, kind="ExternalInput")
    wqk_d = nc.dram_tensor("wqk", [C, 2 * CL], BF16, kind="ExternalInput")
    wv_d = nc.dram_tensor("wv", [C, CL], BF16, kind="ExternalInput")
    bq_d = nc.dram_tensor("bq", [4, 128, 1], F32, kind="ExternalInput")
    wp_d = nc.dram_tensor("wp", [CL, C], BF16, kind="ExternalInput")
    out_d = nc.dram_tensor("out", [T, C], F32, kind="ExternalOutput")

    with TileContext(nc) as tc:
        for _rep in range(repeat):
            _emit_body(nc, tc, xb_d, wqk_d, wv_d, bq_d, wp_d, out_d)

    _split_multiwait_insts(nc)
    return nc


def _emit_body(nc, tc, xb_d, wqk_d, wv_d, bq_d, wp_d, out_d):
    from contextlib import ExitStack

    with ExitStack() as ctx:
        const = ctx.enter_context(tc.tile_pool(name="const", bufs=1))
        big = ctx.enter_context(tc.tile_pool(name="big", bufs=1))
        xtp = ctx.enter_context(tc.tile_pool(name="xtp", bufs=2))
        e_pool = ctx.enter_context(tc.tile_pool(name="epool", bufs=22))
        ynp = ctx.enter_context(tc.tile_pool(name="ynp", bufs=2))
        rcp = ctx.enter_context(tc.tile_pool(name="rcp", bufs=3))
        osp = ctx.enter_context(tc.tile_pool(name="osp", bufs=3))
        mm_ps = ctx.enter_context(tc.tile_pool(name="mmps", bufs=3, space="PSUM"))
        qk_ps = ctx.enter_context(tc.tile_pool(name="qkps", bufs=2, space="PSUM"))
        qkd_ps = ctx.enter_context(tc.tile_pool(name="qkdps", bufs=1, space="PSUM"))

        # ---- persistent tensors ----
        qkT = big.tile([128, 8, T], BF16)          # m 0-3: q chan blocks, 4-7: k
        vaug = big.tile([128, TCH, HL, 65], BF16)  # v natural + ones col (64)
        yT = big.tile([128, 4, T], BF16)           # chan-major y for c_proj
        wqk_sb = big.tile([128, KC, 2 * CL], BF16)
        wv_sb = big.tile([128, KC, CL], BF16)
        wp_sb = big.tile([128, 4, C], BF16)
        bq_sb = const.tile([128, 4, 1], F32)
        idn = const.tile([128, 128], BF16)
        # Packed causal masks for the two diagonal score pairs: the valid
        # column windows of both halves are packed adjacently, so one exp +
        # one mask-mul covers a whole pair.
        m896 = const.tile([128, 896], BF16)   # [tri512 | tri384]
        m384 = const.tile([128, 384], BF16)   # [tri256 | tri128]

        # ---- startup DMAs (order matters: PE's first work is v then qk) ----
        xTs = [None] * NT

        def emit_xt(c, split=False):
            xt = xtp.tile([128, KC, 512], BF16, tag="xT", name=f"xT{c}")
            if split:  # halves so the first v-block unblocks sooner
                for hh in range(2):
                    nc.sync.dma_start_transpose(
                        out=xt[:, :, hh * 256:(hh + 1) * 256],
                        in_=xb_d.ap()[c * 512 + hh * 256:
                                      c * 512 + (hh + 1) * 256, :],
                    )
            else:
                nc.sync.dma_start_transpose(
                    out=xt, in_=xb_d.ap()[c * 512:(c + 1) * 512, :]
                )
            xTs[c] = xt

        # All DMAs go through the SP queue: the tile scheduler pins
        # cross-queue DMA order with completion semaphores (~2us dead time
        # per pin), while same-queue DMAs pipeline back-to-back.
        xt0 = xtp.tile([128, KC, 512], BF16, tag="xT", name="xT0")
        xTs[0] = xt0
        nc.sync.dma_start_transpose(
            out=xt0[:, :, 0:256], in_=xb_d.ap()[0:256, :]
        )
        nc.sync.dma_start(
            out=wv_sb[:, 0:4, :],
            in_=wv_d.ap()[0:512, :].rearrange("(k p) f -> p k f", p=128),
        )
        nc.sync.dma_start(
            out=wv_sb[:, 4:8, :],
            in_=wv_d.ap()[512:1024, :].rearrange("(k p) f -> p k f", p=128),
        )
        nc.sync.dma_start_transpose(
            out=xt0[:, :, 256:512], in_=xb_d.ap()[256:512, :]
        )
        nc.sync.dma_start(
            out=wqk_sb, in_=wqk_d.ap().rearrange("(k p) f -> p k f", p=128)
        )
        nc.sync.dma_start(out=bq_sb, in_=bq_d.ap().rearrange("a p o -> p a o"))
        # xT(1)/wp are issued later (inside QKV(0)) so their DMA requests
        # cannot jump ahead of wqk on the shared DMA engines.

        # masks / ones (DVE+Pool while DMAs stream)
        for mt, widths in ((m896, (512, 384)), (m384, (256, 128))):
            nc.vector.memset(mt, 1.0)
            off = 0
            for w in widths:
                nc.gpsimd.affine_select(
                    out=mt[:, off:off + w], in_=mt[:, off:off + w],
                    pattern=[[1, w]], compare_op=IS_GE, fill=0.0,
                    base=0, channel_multiplier=-1,
                )
                off += w
        nc.vector.memset(vaug[:, :, :, 64:65], 1.0)
        from concourse.masks import make_identity
        make_identity(nc, idn)

        # ---- work units ----
        # unit order: v blocks first (smaller weight tensor arrives first at
        # startup), then qk m-blocks interleaved q/k so that heads 2l/2l+1
        # (which read q block l and k block 4+l) unblock in head order.
        UNIT_ORDER = [("v", 0), ("v", 1), ("v", 2), ("v", 3),
                      ("m", 0), ("m", 4), ("m", 1), ("m", 5),
                      ("m", 2), ("m", 6), ("m", 3), ("m", 7)]

        def qkv_unit(c, u):
            kind, idx = UNIT_ORDER[u]
            if kind == "v":
                tt = idx
                pv = mm_ps.tile([128, CL], F32, tag="mm", name=f"pv{c}_{tt}")
                for k in range(KC):
                    nc.tensor.matmul(
                        out=pv,
                        lhsT=xTs[c][:, k, tt * 128:(tt + 1) * 128],
                        rhs=wv_sb[:, k, :],
                        start=(k == 0),
                        stop=(k == KC - 1),
                    )
                nc.vector.tensor_copy(
                    out=vaug[:, 4 * c + tt, :, 0:64],
                    in_=pv.rearrange("p (h d) -> p h d", d=64),
                )
            else:
                m = idx
                pq = mm_ps.tile([128, 512], F32, tag="mm", name=f"pq{c}_{m}")
                for k in range(KC):
                    nc.tensor.matmul(
                        out=pq,
                        lhsT=wqk_sb[:, k, m * 128:(m + 1) * 128],
                        rhs=xTs[c][:, k, :],
                        start=(k == 0),
                        stop=(k == KC - 1),
                    )
                dst = qkT[:, m, c * 512:(c + 1) * 512]
                if m < 4:  # q: add per-partition bias
                    nc.vector.tensor_scalar_add(dst, pq, bq_sb[:, m, :])
                else:
                    nc.vector.tensor_copy(out=dst, in_=pq)

        def scores_pair(c, l, pj, e_tiles):
            row = (l % 2) * 64
            qtile = l // 2
            ktile = 4 + l // 2
            j0 = 2 * pj - 4 * c
            los = {0: (0, 128), 2: (256, 384)}.get(j0, None)
            if j0 == 2:
                pqk = qkd_ps.tile([128, 384], F32, tag="qkd",
                                  name=f"pqk{c}_{l}_{pj}")
            else:
                pqk = qk_ps.tile([128, 1024], F32, tag="qk",
                                 name=f"pqk{c}_{l}_{pj}")
            e = e_pool.tile([128, 1024], BF16, tag="e", name=f"e{c}_{l}_{pj}")
            if los:
                # diagonal pairs: both halves' valid windows packed
                # adjacently -> one exp + one mask-mul for the pair
                lo0, lo1 = los
                w0, w1 = 512 - lo0, 512 - lo1
                for h, (lo, base) in enumerate(((lo0, 0), (lo1, w0))):
                    nc.tensor.matmul(
                        out=pqk[:, base:base + 512 - lo],
                        lhsT=qkT[row:row + 64, ktile,
                                 (2 * pj + h) * 128:(2 * pj + h + 1) * 128],
                        rhs=qkT[row:row + 64, qtile,
                                c * 512 + lo:(c + 1) * 512],
                        start=True,
                        stop=True,
                    )
                w = w0 + w1
                nc.scalar.activation(
                    out=e[:, 0:w], in_=pqk[:, 0:w], func=EXP, scale=SCALE,
                )
                nc.vector.tensor_mul(
                    e[:, 0:w], e[:, 0:w], m896 if w == 896 else m384
                )
            else:
                for h in range(2):
                    tk = 2 * pj + h
                    nc.tensor.matmul(
                        out=pqk[:, h * 512:(h + 1) * 512],
                        lhsT=qkT[row:row + 64, ktile,
                                 tk * 128:(tk + 1) * 128],
                        rhs=qkT[row:row + 64, qtile,
                                c * 512:(c + 1) * 512],
                        start=True,
                        stop=True,
                    )
                nc.scalar.activation(out=e, in_=pqk, func=EXP, scale=SCALE)
            e_tiles.append(e)

        yv_tiles = {}

        def av_chain(c, l, qq, e_tiles):
            if l not in yv_tiles:
                yvt = mm_ps.tile([128, 512], F32, tag="mm", name=f"yv{c}_{l}")
                yv_tiles[l] = yvt[:, 0:260].rearrange("p (q e) -> p q e", e=65)
            yv = yv_tiles[l]
            nk = 4 * c + qq + 1
            for tk in range(nk):
                pj, h = tk // 2, tk % 2
                j0 = 2 * pj - 4 * c
                los = {0: (0, 128), 2: (256, 384)}.get(j0, None)
                if los:  # packed diagonal-pair e layout
                    lo0, lo1 = los
                    col = (qq * 128 - lo0) if h == 0 else \
                        (512 - lo0) + (qq * 128 - lo1)
                else:
                    col = h * 512 + qq * 128
                nc.tensor.matmul(
                    out=yv[:, qq, :],
                    lhsT=e_tiles[pj][:, col:col + 128],
                    rhs=vaug[:, tk, l, :],
                    start=(tk == 0),
                    stop=(tk == nk - 1),
                )

        def norm_head(c, l, ynat):
            yv = yv_tiles.pop(l)
            rc = rcp.tile([128, 4], F32, tag="rc", name=f"rc{c}_{l}")
            nc.vector.reciprocal(out=rc, in_=yv[:, :, 64])
            nc.vector.tensor_mul(
                ynat[:, :, l * 64:(l + 1) * 64],
                yv[:, :, 0:64],
                rc.unsqueeze(2).broadcast_to((128, 4, 64)),
            )

        os_tiles = {}

        def proj_unit(c, u):
            tq = 4 * c + u // 2
            oc = u % 2
            if oc == 0:
                os_tiles[tq] = osp.tile([128, 1024], F32, tag="os",
                                        name=f"os{tq}")
            os_ = os_tiles[tq]
            pp = mm_ps.tile([128, 512], F32, tag="mm", name=f"pp{tq}_{oc}")
            for k in range(4):
                nc.tensor.matmul(
                    out=pp,
                    lhsT=yT[:, k, tq * 128:(tq + 1) * 128],
                    rhs=wp_sb[:, k, oc * 512:(oc + 1) * 512],
                    start=(k == 0),
                    stop=(k == 3),
                )
            nc.vector.tensor_copy(out=os_[:, oc * 512:(oc + 1) * 512], in_=pp)
            nc.sync.dma_start(
                out=out_d.ap()[tq * 128:(tq + 1) * 128,
                               oc * 512:(oc + 1) * 512],
                in_=os_[:, oc * 512:(oc + 1) * 512],
            )
            if oc == 1:
                del os_tiles[tq]

        # ---- main pipeline ----
        # QKV(0) first 8 units straight (PE's first work; heads 0-3 of
        # chunk 0 unblock), rest deferred into chunk 0's filler stream.
        # Per chunk c: the scores of head l+1 are interleaved at pair
        # granularity with head l's AV chains and with QKV/proj filler
        # units, so PE never sits in an ACT-paced scores run. proj(1) and
        # proj(2) are both deferred to chunk 3, where ACT exp pressure
        # peaks and PE needs the most filler work.
        for u in range(8):
            qkv_unit(0, u)
            if u == 4:
                emit_xt(1)
            if u == 6:
                nc.sync.dma_start(
                    out=wp_sb,
                    in_=wp_d.ap().rearrange("(k p) f -> p k f", p=128),
                )

        for c in range(NT):
            npairs = 2 * c + 2
            if c + 2 < NT:
                emit_xt(c + 2)
            fillers = deque()
            if c == 0:
                for u in range(8, 12):
                    fillers.append(lambda u=u: qkv_unit(0, u))
            if c + 1 < NT:
                for u in range(12):
                    fillers.append(lambda u=u: qkv_unit(c + 1, u))
            if c == 3:
                for cc in (0, 1, 2):
                    for u in range(8):
                        fillers.append(lambda cc=cc, u=u: proj_unit(cc, u))
            nf = len(fillers)
            total_slots = HL * npairs
            done = [0]

            def tick(slot, nf=nf, total_slots=total_slots, done=done,
                     fillers=fillers):
                want = (nf * slot) // total_slots
                while done[0] < want and fillers:
                    fillers.popleft()()
                    done[0] += 1

            ynat = ynp.tile([128, 4, 512], BF16, tag="ynat", name=f"ynat{c}")
            e_heads = [[] for _ in range(HL)]
            for pj in range(npairs):
                scores_pair(c, 0, pj, e_heads[0])
            for l in range(HL):
                # spread head l's 4 AV chains across head l+1's score pairs
                if l + 1 < HL:
                    av_at = {((qq + 1) * npairs) // 5: qq for qq in range(4)}
                    for pj in range(npairs):
                        scores_pair(c, l + 1, pj, e_heads[l + 1])
                        tick(l * npairs + pj + 1)
                        if pj in av_at:
                            av_chain(c, l, av_at[pj], e_heads[l])
                    for qq in range(4):  # c=0 has only 2 pair slots
                        if qq not in av_at.values():
                            av_chain(c, l, qq, e_heads[l])
                else:
                    for qq in range(4):
                        av_chain(c, l, qq, e_heads[l])
                        tick(l * npairs + ((qq + 1) * npairs) // 4)
                norm_head(c, l, ynat)
                if c == 3 and l % 2 == 1 and l < 7:
                    p = l // 2
                    for qq in range(4):
                        nc.sync.dma_start_transpose(
                            out=yT[:, p,
                                   (4 * c + qq) * 128:(4 * c + qq + 1) * 128],
                            in_=ynat[:, qq, p * 128:(p + 1) * 128],
                        )
                if c == 3 and l == 7:
                    # last pair: PE transposes (PE is idle here; skips the
                    # serial DMA issue latency on the critical tail path)
                    tp = qk_ps.tile([128, 1024], F32, tag="qk",
                                    name="tps3").bitcast(BF16)
                    for qq in range(4):
                        nc.tensor.transpose(
                            out=tp[:, qq * 128:(qq + 1) * 128],
                            in_=ynat[:, qq, 384:512],
                            identity=idn,
                        )
                    for qq in range(4):
                        nc.vector.tensor_copy(
                            out=yT[:, 3, (12 + qq) * 128:(13 + qq) * 128],
                            in_=tp[:, qq * 128:(qq + 1) * 128],
                        )
            while fillers:
                fillers.popleft()()
            if c < 3:
                for qq in range(4):
                    nc.sync.dma_start_transpose(
                        out=yT[:, :, (4 * c + qq) * 128:(4 * c + qq + 1) * 128],
                        in_=ynat[:, qq, :],
                    )
        for u in range(8):
            proj_unit(3, u)


# --------------------------------------------------------------------------
# Cached PJRT execution (reuses the compiled executable across calls).
# --------------------------------------------------------------------------
_CACHE = {}


def _get_runner(repeat=1):
    key = ("runner", repeat)
    if key in _CACHE:
        return _CACHE[key]

    import jax
    from jax.sharding import Mesh, PartitionSpec
    from jax.experimental.shard_map import shard_map
    from concourse import bass2jax

    nc = build_bass(repeat=repeat)
    bass2jax.install_neuronx_cc_hook()

    partition_name = (
        nc.partition_id_tensor.name if nc.partition_id_tensor else None
    )
    in_names, out_names, out_avals, zero_shapes = [], [], [], []
    for alloc in nc.m.functions[0].allocations:
        if not isinstance(alloc, mybir.MemoryLocationSet):
            continue
        name = alloc.memorylocations[0].name
        if alloc.kind == "ExternalInput":
            if name != partition_name:
                in_names.append(name)
        elif alloc.kind == "ExternalOutput":
            shape = tuple(alloc.tensor_shape)
            dtype = mybir.dt.np(alloc.dtype)
            out_names.append(name)
            out_avals.append(jax.core.ShapedArray(shape, dtype))
            zero_shapes.append((shape, dtype))
    n_params = len(in_names)
    n_outs = len(out_avals)
    all_in_names = list(in_names) + list(out_names)
    if partition_name is not None:
        all_in_names.append(partition_name)

    def _body(*args):
        operands = list(args)
        if partition_name is not None:
            operands.append(bass2jax.partition_id_tensor())
        outs = bass2jax._bass_exec_p.bind(
            *operands,
            out_avals=tuple(out_avals),
            in_names=tuple(all_in_names),
            out_names=tuple(out_names),
            lowering_input_output_aliases=(),
            sim_require_finite=True,
            sim_require_nnan=True,
            nc=nc,
        )
        return tuple(outs)

    devices = jax.devices()[:NCORES]
    mesh = Mesh(np.asarray(devices), ("core",))
    in_specs = (PartitionSpec("core"),) * (n_params + n_outs)
    out_specs = (PartitionSpec("core"),) * n_outs
    donate = tuple(range(n_params, n_params + n_outs))
    sharded = jax.jit(
        shard_map(
            _body, mesh=mesh, in_specs=in_specs, out_specs=out_specs,
            check_rep=False,
        ),
        donate_argnums=donate,
        keep_unused=True,
    )

    runner = {
        "sharded": sharded,
        "in_names": in_names,
        "out_names": out_names,
        "zero_shapes": zero_shapes,
        "n_params": n_params,
        "mesh": mesh,
    }
    _CACHE[key] = runner
    return runner


def _make_core_inputs(x, W_attn, b_attn, W_proj):
    """Per-core input dicts (core i: batch i//2, head-group i%2)."""
    import ml_dtypes

    bf16 = ml_dtypes.bfloat16
    x = np.ascontiguousarray(x, dtype=np.float32)
    W_attn = np.ascontiguousarray(W_attn, dtype=np.float32)
    b_attn = np.ascontiguousarray(b_attn, dtype=np.float32)
    W_proj = np.ascontiguousarray(W_proj, dtype=np.float32)

    per_group = []
    for g in range(2):
        s = g * CL
        wqk = np.ascontiguousarray(
            np.concatenate(
                [W_attn[:, s:s + CL], W_attn[:, C + s:C + s + CL]], axis=1
            ).astype(bf16)
        )
        wv = np.ascontiguousarray(W_attn[:, 2 * C + s:2 * C + s + CL].astype(bf16))
        bq = np.ascontiguousarray(b_attn[s:s + CL].reshape(4, 128, 1))
        wp = np.ascontiguousarray(W_proj[s:s + CL, :].astype(bf16))
        per_group.append((wqk, wv, bq, wp))

    in_maps = []
    for core in range(NCORES):
        b_i, g = core // 2, core % 2
        wqk, wv, bq, wp = per_group[g]
        in_maps.append(
            {"xb": np.ascontiguousarray(x[b_i].astype(bf16)), "wqk": wqk,
             "wv": wv, "bq": bq, "wp": wp}
        )
    return in_maps


def run_cores(in_maps, timing_reps=0, repeat=1):
    """Run the SPMD kernel. Returns (list of per-core output dicts, best_ns)."""
    import jax, time

    r = _get_runner(repeat=repeat)
    per_core = [
        [np.asarray(m[name]) for name in r["in_names"]] for m in in_maps
    ]
    concat_in = [
        np.concatenate([per_core[c][i] for c in range(NCORES)], axis=0)
        for i in range(len(r["in_names"]))
    ]

    def zeros():
        return [
            np.zeros((NCORES * s[0], *s[1:]), dt) for (s, dt) in r["zero_shapes"]
        ]

    out_arrs = r["sharded"](*concat_in, *zeros())
    outs_np = [np.asarray(a) for a in out_arrs]

    best_ns = None
    if timing_reps > 0:
        from jax.sharding import NamedSharding, PartitionSpec

        shard = NamedSharding(r["mesh"], PartitionSpec("core"))
        dev_in = [jax.device_put(a, shard) for a in concat_in]
        for a in dev_in:
            a.block_until_ready()
        zsets = []
        for _ in range(timing_reps + 1):
            zs = [jax.device_put(z, shard) for z in zeros()]
            for a in zs:
                a.block_until_ready()
            zsets.append(zs)
        res = r["sharded"](*dev_in, *zsets[0])  # warm
        for a in res:
            a.block_until_ready()
        times = []
        for i in range(timing_reps):
            t0 = time.perf_counter()
            res = r["sharded"](*dev_in, *zsets[i + 1])
            for a in res:
                a.block_until_ready()
            t1 = time.perf_counter()
            times.append(t1 - t0)
        best_ns = int(min(times) * 1e9)

    results = []
    for c in range(NCORES):
        m = {}
        for i, name in enumerate(r["out_names"]):
            full = outs_np[i]
            shape = r["zero_shapes"][i][0]
            m[name] = full.reshape(NCORES, *shape)[c]
        results.append(m)
    return results, best_ns


def kernel(x, W_attn, b_attn, W_proj, b_proj, _timing_reps=0, _return_ns=False):
    x = np.asarray(x, dtype=np.float32)
    W_attn = np.asarray(W_attn, dtype=np.float32)
    b_attn = np.asarray(b_attn, dtype=np.float32)
    W_proj = np.asarray(W_proj, dtype=np.float32)
    b_proj = np.asarray(b_proj, dtype=np.float32)

    in_maps = _make_core_inputs(x, W_attn, b_attn, W_proj)
    results, best_ns = run_cores(in_maps, timing_reps=_timing_reps)

    # v-bias contributes a constant row through the projection
    bias_row = (b_proj + b_attn[2 * C:3 * C] @ W_proj).astype(np.float32)

    out = np.empty((B, T, C), dtype=np.float32)
    for b_i in range(B):
        out[b_i] = results[2 * b_i]["out"] + results[2 * b_i + 1]["out"]
        out[b_i] += bias_row[None, :]
    if _return_ns:
        return out, best_ns
    return out


# revision 6
# speedup vs baseline: 1.0314x; 1.0045x over previous
"""Causal self-attention (B=4, T=2048, C=1024, H=16) on 8 Trainium2 cores.

Sharding: core i handles batch b = i//2 and head-group g = i%2 (8 heads,
512 channels). Host sums the two head-group partials per batch and adds
the bias row (v-bias folded through W_proj; k-bias cancels in softmax;
q-bias added on-device).

Design (~224us/core, vs 332us for the f32r baseline):
- x and the weights arrive bf16; xT is produced by XBAR DMA-transpose
  straight from DRAM (no PE transposes, no DVE copy-backs). All PE
  matmuls run at 1 cycle/row.
- AV matmul in natural layout: out[q 128, 65] per (head, qblock) with a
  fused ones-column producing the softmax denominator as column 64 ->
  F=65 moving rows instead of the yT layout's F=512 per key block
  (halves PE time on AV). Normalization is one reciprocal + one
  free-broadcast multiply per head per chunk.
- y transposed back to chan-major via XBAR DMA-transpose per query
  block; the last chunk's final head-pair is transposed on the (then
  idle) PE instead, shortening the tail.
- Diagonal score pairs compute only valid column windows, packed
  adjacently so one exp + one mask-mul covers a whole pair. The small
  diagonal pair gets its own 1-bank PSUM pool; AV accumulators share
  the matmul-drain pool (frees a PSUM bank for score-tile rotation).
- All c_proj work is deferred into the last attention chunk, where ACT
  exp pressure peaks and PE needs filler work; all DMAs go through the
  SP queue (the tile scheduler pins cross-queue DMA order with
  completion semaphores).
"""

import sys
from collections import deque

import numpy as np

sys.path.insert(0, "/opt/trn_rl_repo")

import concourse.bass as bass  # noqa: E402
import concourse.mybir as mybir  # noqa: E402
from concourse.tile import TileContext  # noqa: E402

F32 = mybir.dt.float32
BF16 = mybir.dt.bfloat16
EXP = mybir.ActivationFunctionType.Exp
IS_GE = mybir.AluOpType.is_ge

B, T, C, H, D = 4, 2048, 1024, 16, 64
NCORES = 8
HL = 8          # heads per core
CL = HL * D     # 512 local channels
KC = C // 128   # 8 contraction chunks
TCH = T // 128  # 16 key blocks of 128
NT = T // 512   # 4 query chunks of 512
SCALE = 1.0 / 8.0  # 1/sqrt(64)


# --------------------------------------------------------------------------
# Workaround: this walrus build accepts only ONE sync-wait per instruction.
# Split extras onto fresh single-wait EventSemaphore instructions.
# --------------------------------------------------------------------------
def _split_multiwait_insts(nc):
    ctr = 0
    for f in nc.m.functions:
        for blk in f.blocks:
            insts = list(blk.instructions)
            new_list = []
            changed = False
            for inst in insts:
                si = inst.sync_info
                if si is not None and len(si.on_wait) > 1:
                    waits = list(si.on_wait)
                    keep_idx = len(waits) - 1
                    for i, w in enumerate(waits):
                        if w.wait_reg is not None:
                            keep_idx = i
                            break
                    for i, w in enumerate(waits):
                        if i == keep_idx:
                            continue
                        ev = mybir.InstEventSemaphore(
                            name=f"evsplit_{ctr}", ins=[], outs=[]
                        )
                        ctr += 1
                        ev.engine = inst.engine
                        ev.sync_info = mybir.SyncInfo(on_wait=[w], on_update=[])
                        new_list.append(ev)
                    inst.sync_info.on_wait = [waits[keep_idx]]
                    changed = True
                new_list.append(inst)
            if changed:
                blk.instructions = new_list


def build_bass(repeat=1):
    nc = bass.Bass("TRN2", target_bir_lowering=False, debug=False)

    xb_d = nc.dram_tensor("xb", [T, C], BF16, kind="ExternalInput")
    wqk_d = nc.dram_tensor("wqk", [C, 2 * CL], BF16, kind="ExternalInput")
    wv_d = nc.dram_tensor("wv", [C, CL], BF16, kind="ExternalInput")
    bq_d = nc.dram_tensor("bq", [4, 128, 1], F32, kind="ExternalInput")
    wp_d = nc.dram_tensor("wp", [CL, C], BF16, kind="ExternalInput")
    out_d = nc.dram_tensor("out", [T, C], F32, kind="ExternalOutput")

    with TileContext(nc) as tc:
        for _rep in range(repeat):
            _emit_body(nc, tc, xb_d, wqk_d, wv_d, bq_d, wp_d, out_d)

    _split_multiwait_insts(nc)
    return nc


def _emit_body(nc, tc, xb_d, wqk_d, wv_d, bq_d, wp_d, out_d):
    from contextlib import ExitStack

    with ExitStack() as ctx:
        const = ctx.enter_context(tc.tile_pool(name="const", bufs=1))
        big = ctx.enter_context(tc.tile_pool(name="big", bufs=1))
        xtp = ctx.enter_context(tc.tile_pool(name="xtp", bufs=2))
        e_pool = ctx.enter_context(tc.tile_pool(name="epool", bufs=22))
        ynp = ctx.enter_context(tc.tile_pool(name="ynp", bufs=2))
        rcp = ctx.enter_context(tc.tile_pool(name="rcp", bufs=3))
        osp = ctx.enter_context(tc.tile_pool(name="osp", bufs=3))
        mm_ps = ctx.enter_context(tc.tile_pool(name="mmps", bufs=3, space="PSUM"))
        qk_ps = ctx.enter_context(tc.tile_pool(name="qkps", bufs=2, space="PSUM"))
        qkd_ps = ctx.enter_context(tc.tile_pool(name="qkdps", bufs=1, space="PSUM"))

        # ---- persistent tensors ----
        qkT = big.tile([128, 8, T], BF16)          # m 0-3: q chan blocks, 4-7: k
        vaug = big.tile([128, TCH, HL, 65], BF16)  # v natural + ones col (64)
        yT = big.tile([128, 4, T], BF16)           # chan-major y for c_proj
        wqk_sb = big.tile([128, KC, 2 * CL], BF16)
        wv_sb = big.tile([128, KC, CL], BF16)
        wp_sb = big.tile([128, 4, C], BF16)
        bq_sb = const.tile([128, 4, 1], F32)
        idn = const.tile([128, 128], BF16)
        # Packed causal masks for the two diagonal score pairs: the valid
        # column windows of both halves are packed adjacently, so one exp +
        # one mask-mul covers a whole pair.
        m896 = const.tile([128, 896], BF16)   # [tri512 | tri384]
        m384 = const.tile([128, 384], BF16)   # [tri256 | tri128]

        # ---- startup DMAs (order matters: PE's first work is v then qk) ----
        xTs = [None] * NT

        def emit_xt(c, split=False):
            xt = xtp.tile([128, KC, 512], BF16, tag="xT", name=f"xT{c}")
            if split:  # halves so the first v-block unblocks sooner
                for hh in range(2):
                    nc.sync.dma_start_transpose(
                        out=xt[:, :, hh * 256:(hh + 1) * 256],
                        in_=xb_d.ap()[c * 512 + hh * 256:
                                      c * 512 + (hh + 1) * 256, :],
                    )
            else:
                nc.sync.dma_start_transpose(
                    out=xt, in_=xb_d.ap()[c * 512:(c + 1) * 512, :]
                )
            xTs[c] = xt

        # All DMAs go through the SP queue: the tile scheduler pins
        # cross-queue DMA order with completion semaphores (~2us dead time
        # per pin), while same-queue DMAs pipeline back-to-back.
        xt0 = xtp.tile([128, KC, 512], BF16, tag="xT", name="xT0")
        xTs[0] = xt0
        nc.sync.dma_start_transpose(
            out=xt0[:, :, 0:256], in_=xb_d.ap()[0:256, :]
        )
        nc.sync.dma_start(
            out=wv_sb[:, 0:4, :],
            in_=wv_d.ap()[0:512, :].rearrange("(k p) f -> p k f", p=128),
        )
        nc.sync.dma_start(
            out=wv_sb[:, 4:8, :],
            in_=wv_d.ap()[512:1024, :].rearrange("(k p) f -> p k f", p=128),
        )
        nc.sync.dma_start_transpose(
            out=xt0[:, :, 256:512], in_=xb_d.ap()[256:512, :]
        )
        nc.sync.dma_start(
            out=wqk_sb[:, 0:4, :],
            in_=wqk_d.ap()[0:512, :].rearrange("(k p) f -> p k f", p=128),
        )
        nc.sync.dma_start(
            out=wqk_sb[:, 4:8, :],
            in_=wqk_d.ap()[512:1024, :].rearrange("(k p) f -> p k f", p=128),
        )
        nc.sync.dma_start(out=bq_sb, in_=bq_d.ap().rearrange("a p o -> p a o"))
        # xT(1)/wp are issued later (inside QKV(0)) so their DMA requests
        # cannot jump ahead of wqk on the shared DMA engines.

        # masks / ones (DVE+Pool while DMAs stream)
        for mt, widths in ((m896, (512, 384)), (m384, (256, 128))):
            nc.vector.memset(mt, 1.0)
            off = 0
            for w in widths:
                nc.gpsimd.affine_select(
                    out=mt[:, off:off + w], in_=mt[:, off:off + w],
                    pattern=[[1, w]], compare_op=IS_GE, fill=0.0,
                    base=0, channel_multiplier=-1,
                )
                off += w
        nc.vector.memset(vaug[:, :, :, 64:65], 1.0)
        from concourse.masks import make_identity
        make_identity(nc, idn)

        # ---- work units ----
        # unit order: v blocks first (smaller weight tensor arrives first at
        # startup), then qk m-blocks interleaved q/k so that heads 2l/2l+1
        # (which read q block l and k block 4+l) unblock in head order.
        UNIT_ORDER = [("v", 0), ("v", 1), ("v", 2), ("v", 3),
                      ("m", 0), ("m", 4), ("m", 1), ("m", 5),
                      ("m", 2), ("m", 6), ("m", 3), ("m", 7)]

        def qkv_unit(c, u):
            kind, idx = UNIT_ORDER[u]
            if kind == "v":
                tt = idx
                pv = mm_ps.tile([128, CL], F32, tag="mm", name=f"pv{c}_{tt}")
                for k in range(KC):
                    nc.tensor.matmul(
                        out=pv,
                        lhsT=xTs[c][:, k, tt * 128:(tt + 1) * 128],
                        rhs=wv_sb[:, k, :],
                        start=(k == 0),
                        stop=(k == KC - 1),
                    )
                nc.vector.tensor_copy(
                    out=vaug[:, 4 * c + tt, :, 0:64],
                    in_=pv.rearrange("p (h d) -> p h d", d=64),
                )
            else:
                m = idx
                pq = mm_ps.tile([128, 512], F32, tag="mm", name=f"pq{c}_{m}")
                for k in range(KC):
                    nc.tensor.matmul(
                        out=pq,
                        lhsT=wqk_sb[:, k, m * 128:(m + 1) * 128],
                        rhs=xTs[c][:, k, :],
                        start=(k == 0),
                        stop=(k == KC - 1),
                    )
                dst = qkT[:, m, c * 512:(c + 1) * 512]
                if m < 4:  # q: add per-partition bias
                    nc.vector.tensor_scalar_add(dst, pq, bq_sb[:, m, :])
                else:
                    nc.vector.tensor_copy(out=dst, in_=pq)

        def scores_pair(c, l, pj, e_tiles):
            row = (l % 2) * 64
            qtile = l // 2
            ktile = 4 + l // 2
            j0 = 2 * pj - 4 * c
            los = {0: (0, 128), 2: (256, 384)}.get(j0, None)
            if j0 == 2:
                pqk = qkd_ps.tile([128, 384], F32, tag="qkd",
                                  name=f"pqk{c}_{l}_{pj}")
            else:
                pqk = qk_ps.tile([128, 1024], F32, tag="qk",
                                 name=f"pqk{c}_{l}_{pj}")
            e = e_pool.tile([128, 1024], BF16, tag="e", name=f"e{c}_{l}_{pj}")
            if los:
                # diagonal pairs: both halves' valid windows packed
                # adjacently -> one exp + one mask-mul for the pair
                lo0, lo1 = los
                w0, w1 = 512 - lo0, 512 - lo1
                for h, (lo, base) in enumerate(((lo0, 0), (lo1, w0))):
                    nc.tensor.matmul(
                        out=pqk[:, base:base + 512 - lo],
                        lhsT=qkT[row:row + 64, ktile,
                                 (2 * pj + h) * 128:(2 * pj + h + 1) * 128],
                        rhs=qkT[row:row + 64, qtile,
                                c * 512 + lo:(c + 1) * 512],
                        start=True,
                        stop=True,
                    )
                w = w0 + w1
                nc.scalar.activation(
                    out=e[:, 0:w], in_=pqk[:, 0:w], func=EXP, scale=SCALE,
                )
                nc.vector.tensor_mul(
                    e[:, 0:w], e[:, 0:w], m896 if w == 896 else m384
                )
            else:
                for h in range(2):
                    tk = 2 * pj + h
                    nc.tensor.matmul(
                        out=pqk[:, h * 512:(h + 1) * 512],
                        lhsT=qkT[row:row + 64, ktile,
                                 tk * 128:(tk + 1) * 128],
                        rhs=qkT[row:row + 64, qtile,
                                c * 512:(c + 1) * 512],
                        start=True,
                        stop=True,
                    )
                nc.scalar.activation(out=e, in_=pqk, func=EXP, scale=SCALE)
            e_tiles.append(e)

        yv_tiles = {}

        def av_chain(c, l, qq, e_tiles):
            if l not in yv_tiles:
                yvt = mm_ps.tile([128, 512], F32, tag="mm", name=f"yv{c}_{l}")
                yv_tiles[l] = yvt[:, 0:260].rearrange("p (q e) -> p q e", e=65)
            yv = yv_tiles[l]
            nk = 4 * c + qq + 1
            for tk in range(nk):
                pj, h = tk // 2, tk % 2
                j0 = 2 * pj - 4 * c
                los = {0: (0, 128), 2: (256, 384)}.get(j0, None)
                if los:  # packed diagonal-pair e layout
                    lo0, lo1 = los
                    col = (qq * 128 - lo0) if h == 0 else \
                        (512 - lo0) + (qq * 128 - lo1)
                else:
                    col = h * 512 + qq * 128
                nc.tensor.matmul(
                    out=yv[:, qq, :],
                    lhsT=e_tiles[pj][:, col:col + 128],
                    rhs=vaug[:, tk, l, :],
                    start=(tk == 0),
                    stop=(tk == nk - 1),
                )

        def norm_head(c, l, ynat):
            yv = yv_tiles.pop(l)
            rc = rcp.tile([128, 4], F32, tag="rc", name=f"rc{c}_{l}")
            nc.vector.reciprocal(out=rc, in_=yv[:, :, 64])
            nc.vector.tensor_mul(
                ynat[:, :, l * 64:(l + 1) * 64],
                yv[:, :, 0:64],
                rc.unsqueeze(2).broadcast_to((128, 4, 64)),
            )

        os_tiles = {}

        def proj_unit(c, u):
            tq = 4 * c + u // 2
            oc = u % 2
            if oc == 0:
                os_tiles[tq] = osp.tile([128, 1024], F32, tag="os",
                                        name=f"os{tq}")
            os_ = os_tiles[tq]
            pp = mm_ps.tile([128, 512], F32, tag="mm", name=f"pp{tq}_{oc}")
            for k in range(4):
                nc.tensor.matmul(
                    out=pp,
                    lhsT=yT[:, k, tq * 128:(tq + 1) * 128],
                    rhs=wp_sb[:, k, oc * 512:(oc + 1) * 512],
                    start=(k == 0),
                    stop=(k == 3),
                )
            nc.vector.tensor_copy(out=os_[:, oc * 512:(oc + 1) * 512], in_=pp)
            nc.sync.dma_start(
                out=out_d.ap()[tq * 128:(tq + 1) * 128,
                               oc * 512:(oc + 1) * 512],
                in_=os_[:, oc * 512:(oc + 1) * 512],
            )
            if oc == 1:
                del os_tiles[tq]

        # ---- main pipeline ----
        # QKV(0) first 8 units straight (PE's first work; heads 0-3 of
        # chunk 0 unblock), rest deferred into chunk 0's filler stream.
        # Per chunk c: the scores of head l+1 are interleaved at pair
        # granularity with head l's AV chains and with QKV/proj filler
        # units, so PE never sits in an ACT-paced scores run. proj(1) and
        # proj(2) are both deferred to chunk 3, where ACT exp pressure
        # peaks and PE needs the most filler work.
        for u in range(8):
            qkv_unit(0, u)
            if u == 4:
                emit_xt(1)
            if u == 6:
                nc.sync.dma_start(
                    out=wp_sb,
                    in_=wp_d.ap().rearrange("(k p) f -> p k f", p=128),
                )

        for c in range(NT):
            npairs = 2 * c + 2
            if c + 2 < NT:
                emit_xt(c + 2)
            fillers = deque()
            if c == 0:
                for u in range(8, 12):
                    fillers.append(lambda u=u: qkv_unit(0, u))
            if c + 1 < NT:
                for u in range(12):
                    fillers.append(lambda u=u: qkv_unit(c + 1, u))
            if c == 3:
                for cc in (0, 1, 2):
                    for u in range(8):
                        fillers.append(lambda cc=cc, u=u: proj_unit(cc, u))
            nf = len(fillers)
            total_slots = HL * npairs
            done = [0]

            def tick(slot, nf=nf, total_slots=total_slots, done=done,
                     fillers=fillers):
                want = (nf * slot) // total_slots
                while done[0] < want and fillers:
                    fillers.popleft()()
                    done[0] += 1

            ynat = ynp.tile([128, 4, 512], BF16, tag="ynat", name=f"ynat{c}")
            e_heads = [[] for _ in range(HL)]
            for pj in range(npairs):
                scores_pair(c, 0, pj, e_heads[0])
            for l in range(HL):
                # spread head l's 4 AV chains across head l+1's score pairs
                if l + 1 < HL:
                    av_at = {((qq + 1) * npairs) // 5: qq for qq in range(4)}
                    for pj in range(npairs):
                        scores_pair(c, l + 1, pj, e_heads[l + 1])
                        tick(l * npairs + pj + 1)
                        if pj in av_at:
                            av_chain(c, l, av_at[pj], e_heads[l])
                    for qq in range(4):  # c=0 has only 2 pair slots
                        if qq not in av_at.values():
                            av_chain(c, l, qq, e_heads[l])
                else:
                    for qq in range(4):
                        av_chain(c, l, qq, e_heads[l])
                        tick(l * npairs + ((qq + 1) * npairs) // 4)
                norm_head(c, l, ynat)
                if c == 3 and l % 2 == 1 and l < 7:
                    p = l // 2
                    for qq in range(4):
                        nc.sync.dma_start_transpose(
                            out=yT[:, p,
                                   (4 * c + qq) * 128:(4 * c + qq + 1) * 128],
                            in_=ynat[:, qq, p * 128:(p + 1) * 128],
                        )
                if c == 3 and l == 7:
                    # last pair: PE transposes (PE is idle here; skips the
                    # serial DMA issue latency on the critical tail path)
                    tp = qk_ps.tile([128, 1024], F32, tag="qk",
                                    name="tps3").bitcast(BF16)
                    for qq in range(4):
                        nc.tensor.transpose(
                            out=tp[:, qq * 128:(qq + 1) * 128],
                            in_=ynat[:, qq, 384:512],
                            identity=idn,
                        )
                    for qq in range(4):
                        nc.vector.tensor_copy(
                            out=yT[:, 3, (12 + qq) * 128:(13 + qq) * 128],
                            in_=tp[:, qq * 128:(qq + 1) * 128],
                        )
            while fillers:
                fillers.popleft()()
            if c < 3:
                for qq in range(4):
                    nc.sync.dma_start_transpose(
                        out=yT[:, :, (4 * c + qq) * 128:(4 * c + qq + 1) * 128],
                        in_=ynat[:, qq, :],
                    )
        for u in range(8):
            proj_unit(3, u)


# --------------------------------------------------------------------------
# Cached PJRT execution (reuses the compiled executable across calls).
# --------------------------------------------------------------------------
_CACHE = {}


def _get_runner(repeat=1):
    key = ("runner", repeat)
    if key in _CACHE:
        return _CACHE[key]

    import jax
    from jax.sharding import Mesh, PartitionSpec
    from jax.experimental.shard_map import shard_map
    from concourse import bass2jax

    nc = build_bass(repeat=repeat)
    bass2jax.install_neuronx_cc_hook()

    partition_name = (
        nc.partition_id_tensor.name if nc.partition_id_tensor else None
    )
    in_names, out_names, out_avals, zero_shapes = [], [], [], []
    for alloc in nc.m.functions[0].allocations:
        if not isinstance(alloc, mybir.MemoryLocationSet):
            continue
        name = alloc.memorylocations[0].name
        if alloc.kind == "ExternalInput":
            if name != partition_name:
                in_names.append(name)
        elif alloc.kind == "ExternalOutput":
            shape = tuple(alloc.tensor_shape)
            dtype = mybir.dt.np(alloc.dtype)
            out_names.append(name)
            out_avals.append(jax.core.ShapedArray(shape, dtype))
            zero_shapes.append((shape, dtype))
    n_params = len(in_names)
    n_outs = len(out_avals)
    all_in_names = list(in_names) + list(out_names)
    if partition_name is not None:
        all_in_names.append(partition_name)

    def _body(*args):
        operands = list(args)
        if partition_name is not None:
            operands.append(bass2jax.partition_id_tensor())
        outs = bass2jax._bass_exec_p.bind(
            *operands,
            out_avals=tuple(out_avals),
            in_names=tuple(all_in_names),
            out_names=tuple(out_names),
            lowering_input_output_aliases=(),
            sim_require_finite=True,
            sim_require_nnan=True,
            nc=nc,
        )
        return tuple(outs)

    devices = jax.devices()[:NCORES]
    mesh = Mesh(np.asarray(devices), ("core",))
    in_specs = (PartitionSpec("core"),) * (n_params + n_outs)
    out_specs = (PartitionSpec("core"),) * n_outs
    donate = tuple(range(n_params, n_params + n_outs))
    sharded = jax.jit(
        shard_map(
            _body, mesh=mesh, in_specs=in_specs, out_specs=out_specs,
            check_rep=False,
        ),
        donate_argnums=donate,
        keep_unused=True,
    )

    runner = {
        "sharded": sharded,
        "in_names": in_names,
        "out_names": out_names,
        "zero_shapes": zero_shapes,
        "n_params": n_params,
        "mesh": mesh,
    }
    _CACHE[key] = runner
    return runner


def _make_core_inputs(x, W_attn, b_attn, W_proj):
    """Per-core input dicts (core i: batch i//2, head-group i%2)."""
    import ml_dtypes

    bf16 = ml_dtypes.bfloat16
    x = np.ascontiguousarray(x, dtype=np.float32)
    W_attn = np.ascontiguousarray(W_attn, dtype=np.float32)
    b_attn = np.ascontiguousarray(b_attn, dtype=np.float32)
    W_proj = np.ascontiguousarray(W_proj, dtype=np.float32)

    per_group = []
    for g in range(2):
        s = g * CL
        wqk = np.ascontiguousarray(
            np.concatenate(
                [W_attn[:, s:s + CL], W_attn[:, C + s:C + s + CL]], axis=1
            ).astype(bf16)
        )
        wv = np.ascontiguousarray(W_attn[:, 2 * C + s:2 * C + s + CL].astype(bf16))
        bq = np.ascontiguousarray(b_attn[s:s + CL].reshape(4, 128, 1))
        wp = np.ascontiguousarray(W_proj[s:s + CL, :].astype(bf16))
        per_group.append((wqk, wv, bq, wp))

    in_maps = []
    for core in range(NCORES):
        b_i, g = core // 2, core % 2
        wqk, wv, bq, wp = per_group[g]
        in_maps.append(
            {"xb": np.ascontiguousarray(x[b_i].astype(bf16)), "wqk": wqk,
             "wv": wv, "bq": bq, "wp": wp}
        )
    return in_maps


def run_cores(in_maps, timing_reps=0, repeat=1):
    """Run the SPMD kernel. Returns (list of per-core output dicts, best_ns)."""
    import jax, time

    r = _get_runner(repeat=repeat)
    per_core = [
        [np.asarray(m[name]) for name in r["in_names"]] for m in in_maps
    ]
    concat_in = [
        np.concatenate([per_core[c][i] for c in range(NCORES)], axis=0)
        for i in range(len(r["in_names"]))
    ]

    def zeros():
        return [
            np.zeros((NCORES * s[0], *s[1:]), dt) for (s, dt) in r["zero_shapes"]
        ]

    out_arrs = r["sharded"](*concat_in, *zeros())
    outs_np = [np.asarray(a) for a in out_arrs]

    best_ns = None
    if timing_reps > 0:
        from jax.sharding import NamedSharding, PartitionSpec

        shard = NamedSharding(r["mesh"], PartitionSpec("core"))
        dev_in = [jax.device_put(a, shard) for a in concat_in]
        for a in dev_in:
            a.block_until_ready()
        zsets = []
        for _ in range(timing_reps + 1):
            zs = [jax.device_put(z, shard) for z in zeros()]
            for a in zs:
                a.block_until_ready()
            zsets.append(zs)
        res = r["sharded"](*dev_in, *zsets[0])  # warm
        for a in res:
            a.block_until_ready()
        times = []
        for i in range(timing_reps):
            t0 = time.perf_counter()
            res = r["sharded"](*dev_in, *zsets[i + 1])
            for a in res:
                a.block_until_ready()
            t1 = time.perf_counter()
            times.append(t1 - t0)
        best_ns = int(min(times) * 1e9)

    results = []
    for c in range(NCORES):
        m = {}
        for i, name in enumerate(r["out_names"]):
            full = outs_np[i]
            shape = r["zero_shapes"][i][0]
            m[name] = full.reshape(NCORES, *shape)[c]
        results.append(m)
    return results, best_ns


def kernel(x, W_attn, b_attn, W_proj, b_proj, _timing_reps=0, _return_ns=False):
    x = np.asarray(x, dtype=np.float32)
    W_attn = np.asarray(W_attn, dtype=np.float32)
    b_attn = np.asarray(b_attn, dtype=np.float32)
    W_proj = np.asarray(W_proj, dtype=np.float32)
    b_proj = np.asarray(b_proj, dtype=np.float32)

    in_maps = _make_core_inputs(x, W_attn, b_attn, W_proj)
    results, best_ns = run_cores(in_maps, timing_reps=_timing_reps)

    # v-bias contributes a constant row through the projection
    bias_row = (b_proj + b_attn[2 * C:3 * C] @ W_proj).astype(np.float32)

    out = np.empty((B, T, C), dtype=np.float32)
    for b_i in range(B):
        out[b_i] = results[2 * b_i]["out"] + results[2 * b_i + 1]["out"]
        out[b_i] += bias_row[None, :]
    if _return_ns:
        return out, best_ns
    return out


# revision 7
# speedup vs baseline: 1.0435x; 1.0117x over previous
"""Causal self-attention (B=4, T=2048, C=1024, H=16) on 8 Trainium2 cores.

Sharding: core i handles batch b = i//2 and head-group g = i%2 (8 heads,
512 channels). Host sums the two head-group partials per batch and adds
the bias row (v-bias folded through W_proj; k-bias cancels in softmax;
q-bias added on-device).

Design (~221us/core, vs 332us for the f32r baseline):
- x and the weights arrive bf16; xT is produced by XBAR DMA-transpose
  straight from DRAM (no PE transposes, no DVE copy-backs). All PE
  matmuls run at 1 cycle/row.
- AV matmul in natural layout: out[q 128, 65] per (head, qblock) with a
  fused ones-column producing the softmax denominator as column 64 ->
  F=65 moving rows instead of the yT layout's F=512 per key block
  (halves PE time on AV). Normalization is one reciprocal + one
  free-broadcast multiply per head per chunk.
- y transposed back to chan-major via XBAR DMA-transpose per query
  block; the last chunk's final head-pair is transposed on the (then
  idle) PE instead, shortening the tail.
- Diagonal score pairs compute only valid column windows, packed
  adjacently so one exp + one mask-mul covers a whole pair. The small
  diagonal pair gets its own 1-bank PSUM pool; AV accumulators share
  the matmul-drain pool (frees a PSUM bank for score-tile rotation).
- All c_proj work is deferred into the last attention chunk, where ACT
  exp pressure peaks and PE needs filler work; all DMAs go through the
  SP queue (the tile scheduler pins cross-queue DMA order with
  completion semaphores).
"""

import sys
from collections import deque

import numpy as np

sys.path.insert(0, "/opt/trn_rl_repo")

import concourse.bass as bass  # noqa: E402
import concourse.mybir as mybir  # noqa: E402
from concourse.tile import TileContext  # noqa: E402

F32 = mybir.dt.float32
BF16 = mybir.dt.bfloat16
EXP = mybir.ActivationFunctionType.Exp
IS_GE = mybir.AluOpType.is_ge

B, T, C, H, D = 4, 2048, 1024, 16, 64
NCORES = 8
HL = 8          # heads per core
CL = HL * D     # 512 local channels
KC = C // 128   # 8 contraction chunks
TCH = T // 128  # 16 key blocks of 128
NT = T // 512   # 4 query chunks of 512
SCALE = 1.0 / 8.0  # 1/sqrt(64)


# --------------------------------------------------------------------------
# Workaround: this walrus build accepts only ONE sync-wait per instruction.
# Split extras onto fresh single-wait EventSemaphore instructions.
# --------------------------------------------------------------------------
def _split_multiwait_insts(nc):
    ctr = 0
    for f in nc.m.functions:
        for blk in f.blocks:
            insts = list(blk.instructions)
            new_list = []
            changed = False
            for inst in insts:
                si = inst.sync_info
                if si is not None and len(si.on_wait) > 1:
                    waits = list(si.on_wait)
                    keep_idx = len(waits) - 1
                    for i, w in enumerate(waits):
                        if w.wait_reg is not None:
                            keep_idx = i
                            break
                    for i, w in enumerate(waits):
                        if i == keep_idx:
                            continue
                        ev = mybir.InstEventSemaphore(
                            name=f"evsplit_{ctr}", ins=[], outs=[]
                        )
                        ctr += 1
                        ev.engine = inst.engine
                        ev.sync_info = mybir.SyncInfo(on_wait=[w], on_update=[])
                        new_list.append(ev)
                    inst.sync_info.on_wait = [waits[keep_idx]]
                    changed = True
                new_list.append(inst)
            if changed:
                blk.instructions = new_list


def build_bass(repeat=1):
    nc = bass.Bass("TRN2", target_bir_lowering=False, debug=False)

    xb_d = nc.dram_tensor("xb", [T, C], BF16, kind="ExternalInput")
    wqk_d = nc.dram_tensor("wqk", [C, 2 * CL], BF16, kind="ExternalInput")
    wv_d = nc.dram_tensor("wv", [C, CL], BF16, kind="ExternalInput")
    bq_d = nc.dram_tensor("bq", [4, 128, 1], F32, kind="ExternalInput")
    wp_d = nc.dram_tensor("wp", [CL, C], BF16, kind="ExternalInput")
    out_d = nc.dram_tensor("out", [T, C], F32, kind="ExternalOutput")

    with TileContext(nc) as tc:
        for _rep in range(repeat):
            _emit_body(nc, tc, xb_d, wqk_d, wv_d, bq_d, wp_d, out_d)

    _split_multiwait_insts(nc)
    return nc


def _emit_body(nc, tc, xb_d, wqk_d, wv_d, bq_d, wp_d, out_d):
    from contextlib import ExitStack

    with ExitStack() as ctx:
        const = ctx.enter_context(tc.tile_pool(name="const", bufs=1))
        big = ctx.enter_context(tc.tile_pool(name="big", bufs=1))
        xtp = ctx.enter_context(tc.tile_pool(name="xtp", bufs=2))
        e_pool = ctx.enter_context(tc.tile_pool(name="epool", bufs=22))
        ynp = ctx.enter_context(tc.tile_pool(name="ynp", bufs=2))
        rcp = ctx.enter_context(tc.tile_pool(name="rcp", bufs=3))
        osp = ctx.enter_context(tc.tile_pool(name="osp", bufs=3))
        mm_ps = ctx.enter_context(tc.tile_pool(name="mmps", bufs=3, space="PSUM"))
        qk_ps = ctx.enter_context(tc.tile_pool(name="qkps", bufs=2, space="PSUM"))
        qkd_ps = ctx.enter_context(tc.tile_pool(name="qkdps", bufs=1, space="PSUM"))

        # ---- persistent tensors ----
        qkT = big.tile([128, 8, T], BF16)          # m 0-3: q chan blocks, 4-7: k
        vaug = big.tile([128, TCH, HL, 65], BF16)  # v natural + ones col (64)
        yT = big.tile([128, 4, T], BF16)           # chan-major y for c_proj
        wqk_sb = big.tile([128, KC, 2 * CL], BF16)
        wv_sb = big.tile([128, KC, CL], BF16)
        wp_sb = big.tile([128, 4, C], BF16)
        bq_sb = const.tile([128, 4, 1], F32)
        idn = const.tile([128, 128], BF16)
        # Packed causal masks for the two diagonal score pairs: the valid
        # column windows of both halves are packed adjacently, so one exp +
        # one mask-mul covers a whole pair.
        m896 = const.tile([128, 896], BF16)   # [tri512 | tri384]
        m384 = const.tile([128, 384], BF16)   # [tri256 | tri128]

        # ---- startup DMAs (order matters: PE's first work is v then qk) ----
        xTs = [None] * NT

        def emit_xt(c, split=False):
            xt = xtp.tile([128, KC, 512], BF16, tag="xT", name=f"xT{c}")
            if split:  # halves so the first v-block unblocks sooner
                for hh in range(2):
                    nc.sync.dma_start_transpose(
                        out=xt[:, :, hh * 256:(hh + 1) * 256],
                        in_=xb_d.ap()[c * 512 + hh * 256:
                                      c * 512 + (hh + 1) * 256, :],
                    )
            else:
                nc.sync.dma_start_transpose(
                    out=xt, in_=xb_d.ap()[c * 512:(c + 1) * 512, :]
                )
            xTs[c] = xt

        # All DMAs go through the SP queue: the tile scheduler pins
        # cross-queue DMA order with completion semaphores (~2us dead time
        # per pin), while same-queue DMAs pipeline back-to-back.
        xt0 = xtp.tile([128, KC, 512], BF16, tag="xT", name="xT0")
        xTs[0] = xt0
        for hh in range(2):
            nc.sync.dma_start_transpose(
                out=xt0[:, :, hh * 128:(hh + 1) * 128],
                in_=xb_d.ap()[hh * 128:(hh + 1) * 128, :],
            )
        for kk in range(4):
            nc.sync.dma_start(
                out=wv_sb[:, 2 * kk:2 * kk + 2, :],
                in_=wv_d.ap()[kk * 256:(kk + 1) * 256, :].rearrange(
                    "(k p) f -> p k f", p=128),
            )
        nc.sync.dma_start_transpose(
            out=xt0[:, :, 256:512], in_=xb_d.ap()[256:512, :]
        )
        for kk in range(4):
            nc.sync.dma_start(
                out=wqk_sb[:, 2 * kk:2 * kk + 2, :],
                in_=wqk_d.ap()[kk * 256:(kk + 1) * 256, :].rearrange(
                    "(k p) f -> p k f", p=128),
            )
        nc.sync.dma_start(out=bq_sb, in_=bq_d.ap().rearrange("a p o -> p a o"))
        # xT(1)/wp are issued later (inside QKV(0)) so their DMA requests
        # cannot jump ahead of wqk on the shared DMA engines.

        # masks / ones (DVE+Pool while DMAs stream)
        for mt, widths in ((m896, (512, 384)), (m384, (256, 128))):
            nc.vector.memset(mt, 1.0)
            off = 0
            for w in widths:
                nc.gpsimd.affine_select(
                    out=mt[:, off:off + w], in_=mt[:, off:off + w],
                    pattern=[[1, w]], compare_op=IS_GE, fill=0.0,
                    base=0, channel_multiplier=-1,
                )
                off += w
        nc.vector.memset(vaug[:, :, :, 64:65], 1.0)
        from concourse.masks import make_identity
        make_identity(nc, idn)

        # ---- work units ----
        # unit order: v blocks first (smaller weight tensor arrives first at
        # startup), then qk m-blocks interleaved q/k so that heads 2l/2l+1
        # (which read q block l and k block 4+l) unblock in head order.
        UNIT_ORDER = [("v", 0), ("v", 1), ("v", 2), ("v", 3),
                      ("m", 0), ("m", 4), ("m", 1), ("m", 5),
                      ("m", 2), ("m", 6), ("m", 3), ("m", 7)]

        def qkv_unit(c, u):
            kind, idx = UNIT_ORDER[u]
            if kind == "v":
                tt = idx
                pv = mm_ps.tile([128, CL], F32, tag="mm", name=f"pv{c}_{tt}")
                for k in range(KC):
                    nc.tensor.matmul(
                        out=pv,
                        lhsT=xTs[c][:, k, tt * 128:(tt + 1) * 128],
                        rhs=wv_sb[:, k, :],
                        start=(k == 0),
                        stop=(k == KC - 1),
                    )
                nc.vector.tensor_copy(
                    out=vaug[:, 4 * c + tt, :, 0:64],
                    in_=pv.rearrange("p (h d) -> p h d", d=64),
                )
            else:
                m = idx
                pq = mm_ps.tile([128, 512], F32, tag="mm", name=f"pq{c}_{m}")
                for k in range(KC):
                    nc.tensor.matmul(
                        out=pq,
                        lhsT=wqk_sb[:, k, m * 128:(m + 1) * 128],
                        rhs=xTs[c][:, k, :],
                        start=(k == 0),
                        stop=(k == KC - 1),
                    )
                dst = qkT[:, m, c * 512:(c + 1) * 512]
                if m < 4:  # q: add per-partition bias
                    if c <= 1:
                        nc.scalar.activation(
                            out=dst, in_=pq,
                            func=mybir.ActivationFunctionType.Identity,
                            bias=bq_sb[:, m, :], scale=1.0,
                        )
                    else:
                        nc.vector.tensor_scalar_add(dst, pq, bq_sb[:, m, :])
                elif c <= 1:
                    nc.scalar.copy(out=dst, in_=pq)
                else:
                    nc.vector.tensor_copy(out=dst, in_=pq)

        def scores_pair(c, l, pj, e_tiles):
            row = (l % 2) * 64
            qtile = l // 2
            ktile = 4 + l // 2
            j0 = 2 * pj - 4 * c
            los = {0: (0, 128), 2: (256, 384)}.get(j0, None)
            if j0 == 2:
                pqk = qkd_ps.tile([128, 384], F32, tag="qkd",
                                  name=f"pqk{c}_{l}_{pj}")
            else:
                pqk = qk_ps.tile([128, 1024], F32, tag="qk",
                                 name=f"pqk{c}_{l}_{pj}")
            e = e_pool.tile([128, 1024], BF16, tag="e", name=f"e{c}_{l}_{pj}")
            if los:
                # diagonal pairs: both halves' valid windows packed
                # adjacently -> one exp + one mask-mul for the pair
                lo0, lo1 = los
                w0, w1 = 512 - lo0, 512 - lo1
                for h, (lo, base) in enumerate(((lo0, 0), (lo1, w0))):
                    nc.tensor.matmul(
                        out=pqk[:, base:base + 512 - lo],
                        lhsT=qkT[row:row + 64, ktile,
                                 (2 * pj + h) * 128:(2 * pj + h + 1) * 128],
                        rhs=qkT[row:row + 64, qtile,
                                c * 512 + lo:(c + 1) * 512],
                        start=True,
                        stop=True,
                    )
                w = w0 + w1
                nc.scalar.activation(
                    out=e[:, 0:w], in_=pqk[:, 0:w], func=EXP, scale=SCALE,
                )
                nc.vector.tensor_mul(
                    e[:, 0:w], e[:, 0:w], m896 if w == 896 else m384
                )
            else:
                for h in range(2):
                    tk = 2 * pj + h
                    nc.tensor.matmul(
                        out=pqk[:, h * 512:(h + 1) * 512],
                        lhsT=qkT[row:row + 64, ktile,
                                 tk * 128:(tk + 1) * 128],
                        rhs=qkT[row:row + 64, qtile,
                                c * 512:(c + 1) * 512],
                        start=True,
                        stop=True,
                    )
                nc.scalar.activation(out=e, in_=pqk, func=EXP, scale=SCALE)
            e_tiles.append(e)

        yv_tiles = {}

        def av_chain(c, l, qq, e_tiles):
            if l not in yv_tiles:
                yvt = mm_ps.tile([128, 512], F32, tag="mm", name=f"yv{c}_{l}")
                yv_tiles[l] = yvt[:, 0:260].rearrange("p (q e) -> p q e", e=65)
            yv = yv_tiles[l]
            nk = 4 * c + qq + 1
            for tk in range(nk):
                pj, h = tk // 2, tk % 2
                j0 = 2 * pj - 4 * c
                los = {0: (0, 128), 2: (256, 384)}.get(j0, None)
                if los:  # packed diagonal-pair e layout
                    lo0, lo1 = los
                    col = (qq * 128 - lo0) if h == 0 else \
                        (512 - lo0) + (qq * 128 - lo1)
                else:
                    col = h * 512 + qq * 128
                nc.tensor.matmul(
                    out=yv[:, qq, :],
                    lhsT=e_tiles[pj][:, col:col + 128],
                    rhs=vaug[:, tk, l, :],
                    start=(tk == 0),
                    stop=(tk == nk - 1),
                )

        def norm_head(c, l, ynat):
            yv = yv_tiles.pop(l)
            rc = rcp.tile([128, 4], F32, tag="rc", name=f"rc{c}_{l}")
            nc.vector.reciprocal(out=rc, in_=yv[:, :, 64])
            nc.vector.tensor_mul(
                ynat[:, :, l * 64:(l + 1) * 64],
                yv[:, :, 0:64],
                rc.unsqueeze(2).broadcast_to((128, 4, 64)),
            )

        os_tiles = {}

        def proj_unit(c, u):
            tq = 4 * c + u // 2
            oc = u % 2
            if oc == 0:
                os_tiles[tq] = osp.tile([128, 1024], F32, tag="os",
                                        name=f"os{tq}")
            os_ = os_tiles[tq]
            pp = mm_ps.tile([128, 512], F32, tag="mm", name=f"pp{tq}_{oc}")
            for k in range(4):
                nc.tensor.matmul(
                    out=pp,
                    lhsT=yT[:, k, tq * 128:(tq + 1) * 128],
                    rhs=wp_sb[:, k, oc * 512:(oc + 1) * 512],
                    start=(k == 0),
                    stop=(k == 3),
                )
            nc.vector.tensor_copy(out=os_[:, oc * 512:(oc + 1) * 512], in_=pp)
            nc.sync.dma_start(
                out=out_d.ap()[tq * 128:(tq + 1) * 128,
                               oc * 512:(oc + 1) * 512],
                in_=os_[:, oc * 512:(oc + 1) * 512],
            )
            if oc == 1:
                del os_tiles[tq]

        # ---- main pipeline ----
        # QKV(0) first 8 units straight (PE's first work; heads 0-3 of
        # chunk 0 unblock), rest deferred into chunk 0's filler stream.
        # Per chunk c: the scores of head l+1 are interleaved at pair
        # granularity with head l's AV chains and with QKV/proj filler
        # units, so PE never sits in an ACT-paced scores run. proj(1) and
        # proj(2) are both deferred to chunk 3, where ACT exp pressure
        # peaks and PE needs the most filler work.
        for u in range(8):
            qkv_unit(0, u)
            if u == 4:
                emit_xt(1)
            if u == 6:
                nc.sync.dma_start(
                    out=wp_sb,
                    in_=wp_d.ap().rearrange("(k p) f -> p k f", p=128),
                )

        for c in range(NT):
            npairs = 2 * c + 2
            if c + 2 < NT:
                emit_xt(c + 2)
            fillers = deque()
            if c == 0:
                for u in range(8, 12):
                    fillers.append(lambda u=u: qkv_unit(0, u))
            if c + 1 < NT:
                for u in range(12):
                    fillers.append(lambda u=u: qkv_unit(c + 1, u))
            if c == 3:
                for cc in (0, 1, 2):
                    for u in range(8):
                        fillers.append(lambda cc=cc, u=u: proj_unit(cc, u))
            nf = len(fillers)
            total_slots = HL * npairs
            done = [0]

            def tick(slot, nf=nf, total_slots=total_slots, done=done,
                     fillers=fillers):
                want = (nf * slot) // total_slots
                while done[0] < want and fillers:
                    fillers.popleft()()
                    done[0] += 1

            ynat = ynp.tile([128, 4, 512], BF16, tag="ynat", name=f"ynat{c}")
            e_heads = [[] for _ in range(HL)]
            for pj in range(npairs):
                scores_pair(c, 0, pj, e_heads[0])
            for l in range(HL):
                # spread head l's 4 AV chains across head l+1's score pairs
                if l + 1 < HL:
                    av_at = {((qq + 1) * npairs) // 5: qq for qq in range(4)}
                    for pj in range(npairs):
                        scores_pair(c, l + 1, pj, e_heads[l + 1])
                        tick(l * npairs + pj + 1)
                        if pj in av_at:
                            av_chain(c, l, av_at[pj], e_heads[l])
                    for qq in range(4):  # c=0 has only 2 pair slots
                        if qq not in av_at.values():
                            av_chain(c, l, qq, e_heads[l])
                elif c == 3:
                    # last head of the last chunk: pipeline recip/norm/PE
                    # transpose per query block right behind each AV chain
                    # so the tail's projections start as early as possible
                    yvt = None
                    tp = qk_ps.tile([128, 1024], F32, tag="qk",
                                    name="tps3").bitcast(BF16)
                    for qq in range(4):
                        av_chain(c, l, qq, e_heads[l])
                        yvt = yv_tiles[l]
                        rcq = rcp.tile([128, 1], F32, tag="rc",
                                       name=f"rcq{qq}")
                        nc.vector.reciprocal(out=rcq, in_=yvt[:, qq, 64:65])
                        nc.vector.tensor_mul(
                            ynat[:, qq, l * 64:(l + 1) * 64],
                            yvt[:, qq, 0:64],
                            rcq.broadcast_to((128, 64)),
                        )
                        nc.tensor.transpose(
                            out=tp[:, qq * 128:(qq + 1) * 128],
                            in_=ynat[:, qq, 384:512],
                            identity=idn,
                        )
                        nc.vector.tensor_copy(
                            out=yT[:, 3, (12 + qq) * 128:(13 + qq) * 128],
                            in_=tp[:, qq * 128:(qq + 1) * 128],
                        )
                    del yv_tiles[l]
                else:
                    for qq in range(4):
                        av_chain(c, l, qq, e_heads[l])
                        tick(l * npairs + ((qq + 1) * npairs) // 4)
                if not (c == 3 and l == 7):
                    norm_head(c, l, ynat)
                if c == 3 and l % 2 == 1 and l < 7:
                    p = l // 2
                    for qq in range(4):
                        nc.sync.dma_start_transpose(
                            out=yT[:, p,
                                   (4 * c + qq) * 128:(4 * c + qq + 1) * 128],
                            in_=ynat[:, qq, p * 128:(p + 1) * 128],
                        )
            while fillers:
                fillers.popleft()()
            if c < 3:
                for qq in range(4):
                    nc.sync.dma_start_transpose(
                        out=yT[:, :, (4 * c + qq) * 128:(4 * c + qq + 1) * 128],
                        in_=ynat[:, qq, :],
                    )
        for u in range(8):
            proj_unit(3, u)


# --------------------------------------------------------------------------
# Cached PJRT execution (reuses the compiled executable across calls).
# --------------------------------------------------------------------------
_CACHE = {}


def _get_runner(repeat=1):
    key = ("runner", repeat)
    if key in _CACHE:
        return _CACHE[key]

    import jax
    from jax.sharding import Mesh, PartitionSpec
    from jax.experimental.shard_map import shard_map
    from concourse import bass2jax

    nc = build_bass(repeat=repeat)
    bass2jax.install_neuronx_cc_hook()

    partition_name = (
        nc.partition_id_tensor.name if nc.partition_id_tensor else None
    )
    in_names, out_names, out_avals, zero_shapes = [], [], [], []
    for alloc in nc.m.functions[0].allocations:
        if not isinstance(alloc, mybir.MemoryLocationSet):
            continue
        name = alloc.memorylocations[0].name
        if alloc.kind == "ExternalInput":
            if name != partition_name:
                in_names.append(name)
        elif alloc.kind == "ExternalOutput":
            shape = tuple(alloc.tensor_shape)
            dtype = mybir.dt.np(alloc.dtype)
            out_names.append(name)
            out_avals.append(jax.core.ShapedArray(shape, dtype))
            zero_shapes.append((shape, dtype))
    n_params = len(in_names)
    n_outs = len(out_avals)
    all_in_names = list(in_names) + list(out_names)
    if partition_name is not None:
        all_in_names.append(partition_name)

    def _body(*args):
        operands = list(args)
        if partition_name is not None:
            operands.append(bass2jax.partition_id_tensor())
        outs = bass2jax._bass_exec_p.bind(
            *operands,
            out_avals=tuple(out_avals),
            in_names=tuple(all_in_names),
            out_names=tuple(out_names),
            lowering_input_output_aliases=(),
            sim_require_finite=True,
            sim_require_nnan=True,
            nc=nc,
        )
        return tuple(outs)

    devices = jax.devices()[:NCORES]
    mesh = Mesh(np.asarray(devices), ("core",))
    in_specs = (PartitionSpec("core"),) * (n_params + n_outs)
    out_specs = (PartitionSpec("core"),) * n_outs
    donate = tuple(range(n_params, n_params + n_outs))
    sharded = jax.jit(
        shard_map(
            _body, mesh=mesh, in_specs=in_specs, out_specs=out_specs,
            check_rep=False,
        ),
        donate_argnums=donate,
        keep_unused=True,
    )

    runner = {
        "sharded": sharded,
        "in_names": in_names,
        "out_names": out_names,
        "zero_shapes": zero_shapes,
        "n_params": n_params,
        "mesh": mesh,
    }
    _CACHE[key] = runner
    return runner


def _make_core_inputs(x, W_attn, b_attn, W_proj):
    """Per-core input dicts (core i: batch i//2, head-group i%2)."""
    import ml_dtypes

    bf16 = ml_dtypes.bfloat16
    x = np.ascontiguousarray(x, dtype=np.float32)
    W_attn = np.ascontiguousarray(W_attn, dtype=np.float32)
    b_attn = np.ascontiguousarray(b_attn, dtype=np.float32)
    W_proj = np.ascontiguousarray(W_proj, dtype=np.float32)

    per_group = []
    for g in range(2):
        s = g * CL
        wqk = np.ascontiguousarray(
            np.concatenate(
                [W_attn[:, s:s + CL], W_attn[:, C + s:C + s + CL]], axis=1
            ).astype(bf16)
        )
        wv = np.ascontiguousarray(W_attn[:, 2 * C + s:2 * C + s + CL].astype(bf16))
        bq = np.ascontiguousarray(b_attn[s:s + CL].reshape(4, 128, 1))
        wp = np.ascontiguousarray(W_proj[s:s + CL, :].astype(bf16))
        per_group.append((wqk, wv, bq, wp))

    in_maps = []
    for core in range(NCORES):
        b_i, g = core // 2, core % 2
        wqk, wv, bq, wp = per_group[g]
        in_maps.append(
            {"xb": np.ascontiguousarray(x[b_i].astype(bf16)), "wqk": wqk,
             "wv": wv, "bq": bq, "wp": wp}
        )
    return in_maps


def run_cores(in_maps, timing_reps=0, repeat=1):
    """Run the SPMD kernel. Returns (list of per-core output dicts, best_ns)."""
    import jax, time

    r = _get_runner(repeat=repeat)
    per_core = [
        [np.asarray(m[name]) for name in r["in_names"]] for m in in_maps
    ]
    concat_in = [
        np.concatenate([per_core[c][i] for c in range(NCORES)], axis=0)
        for i in range(len(r["in_names"]))
    ]

    def zeros():
        return [
            np.zeros((NCORES * s[0], *s[1:]), dt) for (s, dt) in r["zero_shapes"]
        ]

    out_arrs = r["sharded"](*concat_in, *zeros())
    outs_np = [np.asarray(a) for a in out_arrs]

    best_ns = None
    if timing_reps > 0:
        from jax.sharding import NamedSharding, PartitionSpec

        shard = NamedSharding(r["mesh"], PartitionSpec("core"))
        dev_in = [jax.device_put(a, shard) for a in concat_in]
        for a in dev_in:
            a.block_until_ready()
        zsets = []
        for _ in range(timing_reps + 1):
            zs = [jax.device_put(z, shard) for z in zeros()]
            for a in zs:
                a.block_until_ready()
            zsets.append(zs)
        res = r["sharded"](*dev_in, *zsets[0])  # warm
        for a in res:
            a.block_until_ready()
        times = []
        for i in range(timing_reps):
            t0 = time.perf_counter()
            res = r["sharded"](*dev_in, *zsets[i + 1])
            for a in res:
                a.block_until_ready()
            t1 = time.perf_counter()
            times.append(t1 - t0)
        best_ns = int(min(times) * 1e9)

    results = []
    for c in range(NCORES):
        m = {}
        for i, name in enumerate(r["out_names"]):
            full = outs_np[i]
            shape = r["zero_shapes"][i][0]
            m[name] = full.reshape(NCORES, *shape)[c]
        results.append(m)
    return results, best_ns


def kernel(x, W_attn, b_attn, W_proj, b_proj, _timing_reps=0, _return_ns=False):
    x = np.asarray(x, dtype=np.float32)
    W_attn = np.asarray(W_attn, dtype=np.float32)
    b_attn = np.asarray(b_attn, dtype=np.float32)
    W_proj = np.asarray(W_proj, dtype=np.float32)
    b_proj = np.asarray(b_proj, dtype=np.float32)

    in_maps = _make_core_inputs(x, W_attn, b_attn, W_proj)
    results, best_ns = run_cores(in_maps, timing_reps=_timing_reps)

    # v-bias contributes a constant row through the projection
    bias_row = (b_proj + b_attn[2 * C:3 * C] @ W_proj).astype(np.float32)

    out = np.empty((B, T, C), dtype=np.float32)
    for b_i in range(B):
        out[b_i] = results[2 * b_i]["out"] + results[2 * b_i + 1]["out"]
        out[b_i] += bias_row[None, :]
    if _return_ns:
        return out, best_ns
    return out


# revision 8
# speedup vs baseline: 1.0438x; 1.0003x over previous
"""Causal self-attention (B=4, T=2048, C=1024, H=16) on 8 Trainium2 cores.

Sharding: core i handles batch b = i//2 and head-group g = i%2 (8 heads,
512 channels). Host sums the two head-group partials per batch and adds
the bias row (v-bias folded through W_proj; k-bias cancels in softmax;
q-bias added on-device).

Design (~221us/core, vs 332us for the f32r baseline):
- x and the weights arrive bf16; xT is produced by XBAR DMA-transpose
  straight from DRAM (no PE transposes, no DVE copy-backs). All PE
  matmuls run at 1 cycle/row.
- AV matmul in natural layout: out[q 128, 65] per (head, qblock) with a
  fused ones-column producing the softmax denominator as column 64 ->
  F=65 moving rows instead of the yT layout's F=512 per key block
  (halves PE time on AV). Normalization is one reciprocal + one
  free-broadcast multiply per head per chunk.
- y transposed back to chan-major via XBAR DMA-transpose per query
  block; the last chunk's final head-pair is transposed on the (then
  idle) PE instead, shortening the tail.
- Diagonal score pairs compute only valid column windows, packed
  adjacently so one exp + one mask-mul covers a whole pair. The small
  diagonal pair gets its own 1-bank PSUM pool; AV accumulators share
  the matmul-drain pool (frees a PSUM bank for score-tile rotation).
- All c_proj work is deferred into the last attention chunk, where ACT
  exp pressure peaks and PE needs filler work; all DMAs go through the
  SP queue (the tile scheduler pins cross-queue DMA order with
  completion semaphores).
"""

import sys
from collections import deque

import numpy as np

sys.path.insert(0, "/opt/trn_rl_repo")

import concourse.bass as bass  # noqa: E402
import concourse.mybir as mybir  # noqa: E402
from concourse.tile import TileContext  # noqa: E402

F32 = mybir.dt.float32
BF16 = mybir.dt.bfloat16
EXP = mybir.ActivationFunctionType.Exp
IS_GE = mybir.AluOpType.is_ge

B, T, C, H, D = 4, 2048, 1024, 16, 64
NCORES = 8
HL = 8          # heads per core
CL = HL * D     # 512 local channels
KC = C // 128   # 8 contraction chunks
TCH = T // 128  # 16 key blocks of 128
NT = T // 512   # 4 query chunks of 512
SCALE = 1.0 / 8.0  # 1/sqrt(64)


# --------------------------------------------------------------------------
# Workaround: this walrus build accepts only ONE sync-wait per instruction.
# Split extras onto fresh single-wait EventSemaphore instructions.
# --------------------------------------------------------------------------
def _split_multiwait_insts(nc):
    ctr = 0
    for f in nc.m.functions:
        for blk in f.blocks:
            insts = list(blk.instructions)
            new_list = []
            changed = False
            for inst in insts:
                si = inst.sync_info
                if si is not None and len(si.on_wait) > 1:
                    waits = list(si.on_wait)
                    keep_idx = len(waits) - 1
                    for i, w in enumerate(waits):
                        if w.wait_reg is not None:
                            keep_idx = i
                            break
                    for i, w in enumerate(waits):
                        if i == keep_idx:
                            continue
                        ev = mybir.InstEventSemaphore(
                            name=f"evsplit_{ctr}", ins=[], outs=[]
                        )
                        ctr += 1
                        ev.engine = inst.engine
                        ev.sync_info = mybir.SyncInfo(on_wait=[w], on_update=[])
                        new_list.append(ev)
                    inst.sync_info.on_wait = [waits[keep_idx]]
                    changed = True
                new_list.append(inst)
            if changed:
                blk.instructions = new_list


def build_bass(repeat=1):
    nc = bass.Bass("TRN2", target_bir_lowering=False, debug=False)

    xb_d = nc.dram_tensor("xb", [T, C], BF16, kind="ExternalInput")
    wqk_d = nc.dram_tensor("wqk", [C, 2 * CL], BF16, kind="ExternalInput")
    wv_d = nc.dram_tensor("wv", [C, CL], BF16, kind="ExternalInput")
    bq_d = nc.dram_tensor("bq", [4, 128, 1], F32, kind="ExternalInput")
    wp_d = nc.dram_tensor("wp", [CL, C], BF16, kind="ExternalInput")
    out_d = nc.dram_tensor("out", [T, C], F32, kind="ExternalOutput")

    with TileContext(nc) as tc:
        for _rep in range(repeat):
            _emit_body(nc, tc, xb_d, wqk_d, wv_d, bq_d, wp_d, out_d)

    _split_multiwait_insts(nc)
    return nc


def _emit_body(nc, tc, xb_d, wqk_d, wv_d, bq_d, wp_d, out_d):
    from contextlib import ExitStack

    with ExitStack() as ctx:
        const = ctx.enter_context(tc.tile_pool(name="const", bufs=1))
        big = ctx.enter_context(tc.tile_pool(name="big", bufs=1))
        xtp = ctx.enter_context(tc.tile_pool(name="xtp", bufs=2))
        e_pool = ctx.enter_context(tc.tile_pool(name="epool", bufs=22))
        ynp = ctx.enter_context(tc.tile_pool(name="ynp", bufs=2))
        rcp = ctx.enter_context(tc.tile_pool(name="rcp", bufs=3))
        osp = ctx.enter_context(tc.tile_pool(name="osp", bufs=3))
        mm_ps = ctx.enter_context(tc.tile_pool(name="mmps", bufs=3, space="PSUM"))
        qk_ps = ctx.enter_context(tc.tile_pool(name="qkps", bufs=2, space="PSUM"))
        qkd_ps = ctx.enter_context(tc.tile_pool(name="qkdps", bufs=1, space="PSUM"))

        # ---- persistent tensors ----
        qkT = big.tile([128, 8, T], BF16)          # m 0-3: q chan blocks, 4-7: k
        vaug = big.tile([128, TCH, HL, 65], BF16)  # v natural + ones col (64)
        yT = big.tile([128, 4, T], BF16)           # chan-major y for c_proj
        wqk_sb = big.tile([128, KC, 2 * CL], BF16)
        wv_sb = big.tile([128, KC, CL], BF16)
        wp_sb = big.tile([128, 4, C], BF16)
        bq_sb = const.tile([128, 4, 1], F32)
        idn = const.tile([128, 128], BF16)
        # Packed causal masks for the two diagonal score pairs: the valid
        # column windows of both halves are packed adjacently, so one exp +
        # one mask-mul covers a whole pair.
        m896 = const.tile([128, 896], BF16)   # [tri512 | tri384]
        m384 = const.tile([128, 384], BF16)   # [tri256 | tri128]

        # ---- startup DMAs (order matters: PE's first work is v then qk) ----
        xTs = [None] * NT

        def emit_xt(c, split=False):
            xt = xtp.tile([128, KC, 512], BF16, tag="xT", name=f"xT{c}")
            if split:  # halves so the first v-block unblocks sooner
                for hh in range(2):
                    nc.sync.dma_start_transpose(
                        out=xt[:, :, hh * 256:(hh + 1) * 256],
                        in_=xb_d.ap()[c * 512 + hh * 256:
                                      c * 512 + (hh + 1) * 256, :],
                    )
            else:
                nc.sync.dma_start_transpose(
                    out=xt, in_=xb_d.ap()[c * 512:(c + 1) * 512, :]
                )
            xTs[c] = xt

        # All DMAs go through the SP queue: the tile scheduler pins
        # cross-queue DMA order with completion semaphores (~2us dead time
        # per pin), while same-queue DMAs pipeline back-to-back.
        xt0 = xtp.tile([128, KC, 512], BF16, tag="xT", name="xT0")
        xTs[0] = xt0
        for hh in range(2):
            nc.sync.dma_start_transpose(
                out=xt0[:, :, hh * 128:(hh + 1) * 128],
                in_=xb_d.ap()[hh * 128:(hh + 1) * 128, :],
            )
        for kk in range(4):
            nc.sync.dma_start(
                out=wv_sb[:, 2 * kk:2 * kk + 2, :],
                in_=wv_d.ap()[kk * 256:(kk + 1) * 256, :].rearrange(
                    "(k p) f -> p k f", p=128),
            )
        nc.sync.dma_start_transpose(
            out=xt0[:, :, 256:512], in_=xb_d.ap()[256:512, :]
        )
        for kk in range(4):
            nc.sync.dma_start(
                out=wqk_sb[:, 2 * kk:2 * kk + 2, :],
                in_=wqk_d.ap()[kk * 256:(kk + 1) * 256, :].rearrange(
                    "(k p) f -> p k f", p=128),
            )
        nc.sync.dma_start(out=bq_sb, in_=bq_d.ap().rearrange("a p o -> p a o"))
        # xT(1)/wp are issued later (inside QKV(0)) so their DMA requests
        # cannot jump ahead of wqk on the shared DMA engines.

        # masks / ones (DVE+Pool while DMAs stream)
        for mt, widths in ((m896, (512, 384)), (m384, (256, 128))):
            nc.vector.memset(mt, 1.0)
            off = 0
            for w in widths:
                nc.gpsimd.affine_select(
                    out=mt[:, off:off + w], in_=mt[:, off:off + w],
                    pattern=[[1, w]], compare_op=IS_GE, fill=0.0,
                    base=0, channel_multiplier=-1,
                )
                off += w
        nc.vector.memset(vaug[:, :, :, 64:65], 1.0)
        from concourse.masks import make_identity
        make_identity(nc, idn)

        # ---- work units ----
        # unit order: v blocks first (smaller weight tensor arrives first at
        # startup), then qk m-blocks interleaved q/k so that heads 2l/2l+1
        # (which read q block l and k block 4+l) unblock in head order.
        UNIT_ORDER = [("v", 0), ("v", 1), ("v", 2), ("v", 3),
                      ("m", 0), ("m", 4), ("m", 1), ("m", 5),
                      ("m", 2), ("m", 6), ("m", 3), ("m", 7)]

        def qkv_unit(c, u):
            kind, idx = UNIT_ORDER[u]
            if kind == "v":
                tt = idx
                pv = mm_ps.tile([128, CL], F32, tag="mm", name=f"pv{c}_{tt}")
                for k in range(KC):
                    nc.tensor.matmul(
                        out=pv,
                        lhsT=xTs[c][:, k, tt * 128:(tt + 1) * 128],
                        rhs=wv_sb[:, k, :],
                        start=(k == 0),
                        stop=(k == KC - 1),
                    )
                nc.vector.tensor_copy(
                    out=vaug[:, 4 * c + tt, :, 0:64],
                    in_=pv.rearrange("p (h d) -> p h d", d=64),
                )
            else:
                m = idx
                pq = mm_ps.tile([128, 512], F32, tag="mm", name=f"pq{c}_{m}")
                for k in range(KC):
                    nc.tensor.matmul(
                        out=pq,
                        lhsT=wqk_sb[:, k, m * 128:(m + 1) * 128],
                        rhs=xTs[c][:, k, :],
                        start=(k == 0),
                        stop=(k == KC - 1),
                    )
                dst = qkT[:, m, c * 512:(c + 1) * 512]
                if m < 4:  # q: add per-partition bias
                    if c <= 1:
                        nc.scalar.activation(
                            out=dst, in_=pq,
                            func=mybir.ActivationFunctionType.Identity,
                            bias=bq_sb[:, m, :], scale=1.0,
                        )
                    else:
                        nc.vector.tensor_scalar_add(dst, pq, bq_sb[:, m, :])
                elif c <= 1:
                    nc.scalar.copy(out=dst, in_=pq)
                else:
                    nc.vector.tensor_copy(out=dst, in_=pq)

        def scores_pair(c, l, pj, e_tiles):
            row = (l % 2) * 64
            qtile = l // 2
            ktile = 4 + l // 2
            j0 = 2 * pj - 4 * c
            los = {0: (0, 128), 2: (256, 384)}.get(j0, None)
            if j0 == 2:
                pqk = qkd_ps.tile([128, 384], F32, tag="qkd",
                                  name=f"pqk{c}_{l}_{pj}")
            else:
                pqk = qk_ps.tile([128, 1024], F32, tag="qk",
                                 name=f"pqk{c}_{l}_{pj}")
            e = e_pool.tile([128, 1024], BF16, tag="e", name=f"e{c}_{l}_{pj}")
            if los:
                # diagonal pairs: both halves' valid windows packed
                # adjacently -> one exp + one mask-mul for the pair
                lo0, lo1 = los
                w0, w1 = 512 - lo0, 512 - lo1
                for h, (lo, base) in enumerate(((lo0, 0), (lo1, w0))):
                    nc.tensor.matmul(
                        out=pqk[:, base:base + 512 - lo],
                        lhsT=qkT[row:row + 64, ktile,
                                 (2 * pj + h) * 128:(2 * pj + h + 1) * 128],
                        rhs=qkT[row:row + 64, qtile,
                                c * 512 + lo:(c + 1) * 512],
                        start=True,
                        stop=True,
                    )
                w = w0 + w1
                nc.scalar.activation(
                    out=e[:, 0:w], in_=pqk[:, 0:w], func=EXP, scale=SCALE,
                )
                if w == 896:
                    # only the two 128-wide diagonal triangle sub-blocks
                    # can be invalid; mask just those
                    tri128 = m384[:, 256:384]
                    nc.vector.tensor_mul(e[:, 0:128], e[:, 0:128], tri128)
                    nc.vector.tensor_mul(e[:, 512:640], e[:, 512:640], tri128)
                else:
                    nc.vector.tensor_mul(e[:, 0:w], e[:, 0:w], m384)
            else:
                for h in range(2):
                    tk = 2 * pj + h
                    nc.tensor.matmul(
                        out=pqk[:, h * 512:(h + 1) * 512],
                        lhsT=qkT[row:row + 64, ktile,
                                 tk * 128:(tk + 1) * 128],
                        rhs=qkT[row:row + 64, qtile,
                                c * 512:(c + 1) * 512],
                        start=True,
                        stop=True,
                    )
                nc.scalar.activation(out=e, in_=pqk, func=EXP, scale=SCALE)
            e_tiles.append(e)

        yv_tiles = {}

        def av_chain(c, l, qq, e_tiles):
            if l not in yv_tiles:
                yvt = mm_ps.tile([128, 512], F32, tag="mm", name=f"yv{c}_{l}")
                yv_tiles[l] = yvt[:, 0:260].rearrange("p (q e) -> p q e", e=65)
            yv = yv_tiles[l]
            nk = 4 * c + qq + 1
            for tk in range(nk):
                pj, h = tk // 2, tk % 2
                j0 = 2 * pj - 4 * c
                los = {0: (0, 128), 2: (256, 384)}.get(j0, None)
                if los:  # packed diagonal-pair e layout
                    lo0, lo1 = los
                    col = (qq * 128 - lo0) if h == 0 else \
                        (512 - lo0) + (qq * 128 - lo1)
                else:
                    col = h * 512 + qq * 128
                nc.tensor.matmul(
                    out=yv[:, qq, :],
                    lhsT=e_tiles[pj][:, col:col + 128],
                    rhs=vaug[:, tk, l, :],
                    start=(tk == 0),
                    stop=(tk == nk - 1),
                )

        def norm_head(c, l, ynat):
            yv = yv_tiles.pop(l)
            rc = rcp.tile([128, 4], F32, tag="rc", name=f"rc{c}_{l}")
            nc.vector.reciprocal(out=rc, in_=yv[:, :, 64])
            nc.vector.tensor_mul(
                ynat[:, :, l * 64:(l + 1) * 64],
                yv[:, :, 0:64],
                rc.unsqueeze(2).broadcast_to((128, 4, 64)),
            )

        os_tiles = {}

        def proj_unit(c, u):
            tq = 4 * c + u // 2
            oc = u % 2
            if oc == 0:
                os_tiles[tq] = osp.tile([128, 1024], F32, tag="os",
                                        name=f"os{tq}")
            os_ = os_tiles[tq]
            pp = mm_ps.tile([128, 512], F32, tag="mm", name=f"pp{tq}_{oc}")
            for k in range(4):
                nc.tensor.matmul(
                    out=pp,
                    lhsT=yT[:, k, tq * 128:(tq + 1) * 128],
                    rhs=wp_sb[:, k, oc * 512:(oc + 1) * 512],
                    start=(k == 0),
                    stop=(k == 3),
                )
            nc.vector.tensor_copy(out=os_[:, oc * 512:(oc + 1) * 512], in_=pp)
            nc.sync.dma_start(
                out=out_d.ap()[tq * 128:(tq + 1) * 128,
                               oc * 512:(oc + 1) * 512],
                in_=os_[:, oc * 512:(oc + 1) * 512],
            )
            if oc == 1:
                del os_tiles[tq]

        # ---- main pipeline ----
        # QKV(0) first 8 units straight (PE's first work; heads 0-3 of
        # chunk 0 unblock), rest deferred into chunk 0's filler stream.
        # Per chunk c: the scores of head l+1 are interleaved at pair
        # granularity with head l's AV chains and with QKV/proj filler
        # units, so PE never sits in an ACT-paced scores run. proj(1) and
        # proj(2) are both deferred to chunk 3, where ACT exp pressure
        # peaks and PE needs the most filler work.
        for u in range(8):
            qkv_unit(0, u)
            if u == 4:
                emit_xt(1)
            if u == 6:
                nc.sync.dma_start(
                    out=wp_sb,
                    in_=wp_d.ap().rearrange("(k p) f -> p k f", p=128),
                )

        for c in range(NT):
            npairs = 2 * c + 2
            if c + 2 < NT:
                emit_xt(c + 2)
            fillers = deque()
            if c == 0:
                for u in range(8, 12):
                    fillers.append(lambda u=u: qkv_unit(0, u))
            if c + 1 < NT:
                for u in range(12):
                    fillers.append(lambda u=u: qkv_unit(c + 1, u))
            if c == 3:
                for cc in (0, 1, 2):
                    for u in range(8):
                        fillers.append(lambda cc=cc, u=u: proj_unit(cc, u))
            nf = len(fillers)
            total_slots = HL * npairs
            done = [0]

            def tick(slot, nf=nf, total_slots=total_slots, done=done,
                     fillers=fillers):
                want = (nf * slot) // total_slots
                while done[0] < want and fillers:
                    fillers.popleft()()
                    done[0] += 1

            ynat = ynp.tile([128, 4, 512], BF16, tag="ynat", name=f"ynat{c}")
            e_heads = [[] for _ in range(HL)]
            for pj in range(npairs):
                scores_pair(c, 0, pj, e_heads[0])
            for l in range(HL):
                # spread head l's 4 AV chains across head l+1's score pairs
                if l + 1 < HL:
                    av_at = {((qq + 1) * npairs) // 5: qq for qq in range(4)}
                    for pj in range(npairs):
                        scores_pair(c, l + 1, pj, e_heads[l + 1])
                        tick(l * npairs + pj + 1)
                        if pj in av_at:
                            av_chain(c, l, av_at[pj], e_heads[l])
                    for qq in range(4):  # c=0 has only 2 pair slots
                        if qq not in av_at.values():
                            av_chain(c, l, qq, e_heads[l])
                elif c == 3:
                    # last head of the last chunk: pipeline recip/norm/PE
                    # transpose per query block right behind each AV chain
                    # so the tail's projections start as early as possible
                    yvt = None
                    tp = qk_ps.tile([128, 1024], F32, tag="qk",
                                    name="tps3").bitcast(BF16)
                    for qq in range(4):
                        av_chain(c, l, qq, e_heads[l])
                        yvt = yv_tiles[l]
                        rcq = rcp.tile([128, 1], F32, tag="rc",
                                       name=f"rcq{qq}")
                        nc.vector.reciprocal(out=rcq, in_=yvt[:, qq, 64:65])
                        nc.vector.tensor_mul(
                            ynat[:, qq, l * 64:(l + 1) * 64],
                            yvt[:, qq, 0:64],
                            rcq.broadcast_to((128, 64)),
                        )
                        nc.tensor.transpose(
                            out=tp[:, qq * 128:(qq + 1) * 128],
                            in_=ynat[:, qq, 384:512],
                            identity=idn,
                        )
                        nc.vector.tensor_copy(
                            out=yT[:, 3, (12 + qq) * 128:(13 + qq) * 128],
                            in_=tp[:, qq * 128:(qq + 1) * 128],
                        )
                    del yv_tiles[l]
                else:
                    for qq in range(4):
                        av_chain(c, l, qq, e_heads[l])
                        tick(l * npairs + ((qq + 1) * npairs) // 4)
                if not (c == 3 and l == 7):
                    norm_head(c, l, ynat)
                if c == 3 and l % 2 == 1 and l < 7:
                    p = l // 2
                    for qq in range(4):
                        nc.sync.dma_start_transpose(
                            out=yT[:, p,
                                   (4 * c + qq) * 128:(4 * c + qq + 1) * 128],
                            in_=ynat[:, qq, p * 128:(p + 1) * 128],
                        )
            while fillers:
                fillers.popleft()()
            if c < 3:
                for qq in range(4):
                    nc.sync.dma_start_transpose(
                        out=yT[:, :, (4 * c + qq) * 128:(4 * c + qq + 1) * 128],
                        in_=ynat[:, qq, :],
                    )
        for u in range(8):
            proj_unit(3, u)


# --------------------------------------------------------------------------
# Cached PJRT execution (reuses the compiled executable across calls).
# --------------------------------------------------------------------------
_CACHE = {}


def _get_runner(repeat=1):
    key = ("runner", repeat)
    if key in _CACHE:
        return _CACHE[key]

    import jax
    from jax.sharding import Mesh, PartitionSpec
    from jax.experimental.shard_map import shard_map
    from concourse import bass2jax

    nc = build_bass(repeat=repeat)
    bass2jax.install_neuronx_cc_hook()

    partition_name = (
        nc.partition_id_tensor.name if nc.partition_id_tensor else None
    )
    in_names, out_names, out_avals, zero_shapes = [], [], [], []
    for alloc in nc.m.functions[0].allocations:
        if not isinstance(alloc, mybir.MemoryLocationSet):
            continue
        name = alloc.memorylocations[0].name
        if alloc.kind == "ExternalInput":
            if name != partition_name:
                in_names.append(name)
        elif alloc.kind == "ExternalOutput":
            shape = tuple(alloc.tensor_shape)
            dtype = mybir.dt.np(alloc.dtype)
            out_names.append(name)
            out_avals.append(jax.core.ShapedArray(shape, dtype))
            zero_shapes.append((shape, dtype))
    n_params = len(in_names)
    n_outs = len(out_avals)
    all_in_names = list(in_names) + list(out_names)
    if partition_name is not None:
        all_in_names.append(partition_name)

    def _body(*args):
        operands = list(args)
        if partition_name is not None:
            operands.append(bass2jax.partition_id_tensor())
        outs = bass2jax._bass_exec_p.bind(
            *operands,
            out_avals=tuple(out_avals),
            in_names=tuple(all_in_names),
            out_names=tuple(out_names),
            lowering_input_output_aliases=(),
            sim_require_finite=True,
            sim_require_nnan=True,
            nc=nc,
        )
        return tuple(outs)

    devices = jax.devices()[:NCORES]
    mesh = Mesh(np.asarray(devices), ("core",))
    in_specs = (PartitionSpec("core"),) * (n_params + n_outs)
    out_specs = (PartitionSpec("core"),) * n_outs
    donate = tuple(range(n_params, n_params + n_outs))
    sharded = jax.jit(
        shard_map(
            _body, mesh=mesh, in_specs=in_specs, out_specs=out_specs,
            check_rep=False,
        ),
        donate_argnums=donate,
        keep_unused=True,
    )

    runner = {
        "sharded": sharded,
        "in_names": in_names,
        "out_names": out_names,
        "zero_shapes": zero_shapes,
        "n_params": n_params,
        "mesh": mesh,
    }
    _CACHE[key] = runner
    return runner


def _make_core_inputs(x, W_attn, b_attn, W_proj):
    """Per-core input dicts (core i: batch i//2, head-group i%2)."""
    import ml_dtypes

    bf16 = ml_dtypes.bfloat16
    x = np.ascontiguousarray(x, dtype=np.float32)
    W_attn = np.ascontiguousarray(W_attn, dtype=np.float32)
    b_attn = np.ascontiguousarray(b_attn, dtype=np.float32)
    W_proj = np.ascontiguousarray(W_proj, dtype=np.float32)

    per_group = []
    for g in range(2):
        s = g * CL
        wqk = np.ascontiguousarray(
            np.concatenate(
                [W_attn[:, s:s + CL], W_attn[:, C + s:C + s + CL]], axis=1
            ).astype(bf16)
        )
        wv = np.ascontiguousarray(W_attn[:, 2 * C + s:2 * C + s + CL].astype(bf16))
        bq = np.ascontiguousarray(b_attn[s:s + CL].reshape(4, 128, 1))
        wp = np.ascontiguousarray(W_proj[s:s + CL, :].astype(bf16))
        per_group.append((wqk, wv, bq, wp))

    in_maps = []
    for core in range(NCORES):
        b_i, g = core // 2, core % 2
        wqk, wv, bq, wp = per_group[g]
        in_maps.append(
            {"xb": np.ascontiguousarray(x[b_i].astype(bf16)), "wqk": wqk,
             "wv": wv, "bq": bq, "wp": wp}
        )
    return in_maps


def run_cores(in_maps, timing_reps=0, repeat=1):
    """Run the SPMD kernel. Returns (list of per-core output dicts, best_ns)."""
    import jax, time

    r = _get_runner(repeat=repeat)
    per_core = [
        [np.asarray(m[name]) for name in r["in_names"]] for m in in_maps
    ]
    concat_in = [
        np.concatenate([per_core[c][i] for c in range(NCORES)], axis=0)
        for i in range(len(r["in_names"]))
    ]

    def zeros():
        return [
            np.zeros((NCORES * s[0], *s[1:]), dt) for (s, dt) in r["zero_shapes"]
        ]

    out_arrs = r["sharded"](*concat_in, *zeros())
    outs_np = [np.asarray(a) for a in out_arrs]

    best_ns = None
    if timing_reps > 0:
        from jax.sharding import NamedSharding, PartitionSpec

        shard = NamedSharding(r["mesh"], PartitionSpec("core"))
        dev_in = [jax.device_put(a, shard) for a in concat_in]
        for a in dev_in:
            a.block_until_ready()
        zsets = []
        for _ in range(timing_reps + 1):
            zs = [jax.device_put(z, shard) for z in zeros()]
            for a in zs:
                a.block_until_ready()
            zsets.append(zs)
        res = r["sharded"](*dev_in, *zsets[0])  # warm
        for a in res:
            a.block_until_ready()
        times = []
        for i in range(timing_reps):
            t0 = time.perf_counter()
            res = r["sharded"](*dev_in, *zsets[i + 1])
            for a in res:
                a.block_until_ready()
            t1 = time.perf_counter()
            times.append(t1 - t0)
        best_ns = int(min(times) * 1e9)

    results = []
    for c in range(NCORES):
        m = {}
        for i, name in enumerate(r["out_names"]):
            full = outs_np[i]
            shape = r["zero_shapes"][i][0]
            m[name] = full.reshape(NCORES, *shape)[c]
        results.append(m)
    return results, best_ns


def kernel(x, W_attn, b_attn, W_proj, b_proj, _timing_reps=0, _return_ns=False):
    x = np.asarray(x, dtype=np.float32)
    W_attn = np.asarray(W_attn, dtype=np.float32)
    b_attn = np.asarray(b_attn, dtype=np.float32)
    W_proj = np.asarray(W_proj, dtype=np.float32)
    b_proj = np.asarray(b_proj, dtype=np.float32)

    in_maps = _make_core_inputs(x, W_attn, b_attn, W_proj)
    results, best_ns = run_cores(in_maps, timing_reps=_timing_reps)

    # v-bias contributes a constant row through the projection
    bias_row = (b_proj + b_attn[2 * C:3 * C] @ W_proj).astype(np.float32)

    out = np.empty((B, T, C), dtype=np.float32)
    for b_i in range(B):
        out[b_i] = results[2 * b_i]["out"] + results[2 * b_i + 1]["out"]
        out[b_i] += bias_row[None, :]
    if _return_ns:
        return out, best_ns
    return out


# revision 11
# speedup vs baseline: 1.0608x; 1.0163x over previous
"""Causal self-attention (B=4, T=2048, C=1024, H=16) on 8 Trainium2 cores.

Sharding: core i handles batch b = i//2 and head-group g = i%2 (8 heads,
512 channels). Host sums the two head-group partials per batch and adds
the bias row (v-bias folded through W_proj; k-bias cancels in softmax;
q-bias added on-device).

Design (~221us/core, vs 332us for the f32r baseline):
- x and the weights arrive bf16; xT is produced by XBAR DMA-transpose
  straight from DRAM (no PE transposes, no DVE copy-backs). All PE
  matmuls run at 1 cycle/row.
- AV matmul in natural layout: out[q 128, 65] per (head, qblock) with a
  fused ones-column producing the softmax denominator as column 64 ->
  F=65 moving rows instead of the yT layout's F=512 per key block
  (halves PE time on AV). Normalization is one reciprocal + one
  free-broadcast multiply per head per chunk.
- y transposed back to chan-major via XBAR DMA-transpose per query
  block; the last chunk's final head-pair is transposed on the (then
  idle) PE instead, shortening the tail.
- Diagonal score pairs compute only valid column windows, packed
  adjacently so one exp + one mask-mul covers a whole pair. The small
  diagonal pair gets its own 1-bank PSUM pool; AV accumulators share
  the matmul-drain pool (frees a PSUM bank for score-tile rotation).
- All c_proj work is deferred into the last attention chunk, where ACT
  exp pressure peaks and PE needs filler work; all DMAs go through the
  SP queue (the tile scheduler pins cross-queue DMA order with
  completion semaphores).
"""

import sys
from collections import deque

import numpy as np

sys.path.insert(0, "/opt/trn_rl_repo")

import concourse.bass as bass  # noqa: E402
import concourse.mybir as mybir  # noqa: E402
from concourse.tile import TileContext  # noqa: E402

F32 = mybir.dt.float32
BF16 = mybir.dt.bfloat16
EXP = mybir.ActivationFunctionType.Exp
IS_GE = mybir.AluOpType.is_ge

B, T, C, H, D = 4, 2048, 1024, 16, 64
NCORES = 8
HL = 8          # heads per core
CL = HL * D     # 512 local channels
KC = C // 128   # 8 contraction chunks
TCH = T // 128  # 16 key blocks of 128
NT = T // 512   # 4 query chunks of 512
SCALE = 1.0 / 8.0  # 1/sqrt(64)


# --------------------------------------------------------------------------
# Workaround: this walrus build accepts only ONE sync-wait per instruction.
# Split extras onto fresh single-wait EventSemaphore instructions.
# --------------------------------------------------------------------------
def _split_multiwait_insts(nc):
    ctr = 0
    for f in nc.m.functions:
        for blk in f.blocks:
            insts = list(blk.instructions)
            new_list = []
            changed = False
            for inst in insts:
                si = inst.sync_info
                if si is not None and len(si.on_wait) > 1:
                    waits = list(si.on_wait)
                    keep_idx = len(waits) - 1
                    for i, w in enumerate(waits):
                        if w.wait_reg is not None:
                            keep_idx = i
                            break
                    for i, w in enumerate(waits):
                        if i == keep_idx:
                            continue
                        ev = mybir.InstEventSemaphore(
                            name=f"evsplit_{ctr}", ins=[], outs=[]
                        )
                        ctr += 1
                        ev.engine = inst.engine
                        ev.sync_info = mybir.SyncInfo(on_wait=[w], on_update=[])
                        new_list.append(ev)
                    inst.sync_info.on_wait = [waits[keep_idx]]
                    changed = True
                new_list.append(inst)
            if changed:
                blk.instructions = new_list


def build_bass(repeat=1):
    nc = bass.Bass("TRN2", target_bir_lowering=False, debug=False)

    xb_d = nc.dram_tensor("xb", [T, C], BF16, kind="ExternalInput")
    wqk_d = nc.dram_tensor("wqk", [C, 2 * CL], BF16, kind="ExternalInput")
    wv_d = nc.dram_tensor("wv", [C, CL], BF16, kind="ExternalInput")
    bq_d = nc.dram_tensor("bq", [4, 128, 1], F32, kind="ExternalInput")
    wp_d = nc.dram_tensor("wp", [CL, C], BF16, kind="ExternalInput")
    out_d = nc.dram_tensor("out", [T, C], F32, kind="ExternalOutput")

    with TileContext(nc) as tc:
        for _rep in range(repeat):
            _emit_body(nc, tc, xb_d, wqk_d, wv_d, bq_d, wp_d, out_d)

    _split_multiwait_insts(nc)
    return nc


def _emit_body(nc, tc, xb_d, wqk_d, wv_d, bq_d, wp_d, out_d):
    from contextlib import ExitStack

    with ExitStack() as ctx:
        const = ctx.enter_context(tc.tile_pool(name="const", bufs=1))
        big = ctx.enter_context(tc.tile_pool(name="big", bufs=1))
        xtp = ctx.enter_context(tc.tile_pool(name="xtp", bufs=2))
        e_pool = ctx.enter_context(tc.tile_pool(name="epool", bufs=22))
        ynp = ctx.enter_context(tc.tile_pool(name="ynp", bufs=2))
        rcp = ctx.enter_context(tc.tile_pool(name="rcp", bufs=3))
        osp = ctx.enter_context(tc.tile_pool(name="osp", bufs=3))
        mm_ps = ctx.enter_context(tc.tile_pool(name="mmps", bufs=3, space="PSUM"))
        qk_ps = ctx.enter_context(tc.tile_pool(name="qkps", bufs=2, space="PSUM"))
        qkd_ps = ctx.enter_context(tc.tile_pool(name="qkdps", bufs=1, space="PSUM"))

        # ---- persistent tensors ----
        qkT = big.tile([128, 8, T], BF16)          # m 0-3: q chan blocks, 4-7: k
        vaug = big.tile([128, TCH, HL, 65], BF16)  # v natural + ones col (64)
        yT = big.tile([128, 4, T], BF16)           # chan-major y for c_proj
        wqk_sb = big.tile([128, KC, 2 * CL], BF16)
        wv_sb = big.tile([128, KC, CL], BF16)
        wp_sb = big.tile([128, 4, C], BF16)
        bq_sb = const.tile([128, 4, 1], F32)
        idn = const.tile([128, 128], BF16)
        # Packed causal masks for the two diagonal score pairs: the valid
        # column windows of both halves are packed adjacently, so one exp +
        # one mask-mul covers a whole pair.
        m896 = const.tile([128, 896], BF16)   # [tri512 | tri384]
        m384 = const.tile([128, 384], BF16)   # [tri256 | tri128]

        # masks / ones (DVE+Pool while DMAs stream)
        for mt, widths in ((m896, (512, 384)), (m384, (256, 128))):
            nc.vector.memset(mt, 1.0)
            off = 0
            for w in widths:
                nc.gpsimd.affine_select(
                    out=mt[:, off:off + w], in_=mt[:, off:off + w],
                    pattern=[[1, w]], compare_op=IS_GE, fill=0.0,
                    base=0, channel_multiplier=-1,
                )
                off += w
        nc.vector.memset(vaug[:, :, :, 64:65], 1.0)
        from concourse.masks import make_identity
        make_identity(nc, idn)

        # ---- startup DMAs (order matters: PE's first work is v then qk) ----
        xTs = [None] * NT

        def emit_xt(c, split=False):
            xt = xtp.tile([128, KC, 512], BF16, tag="xT", name=f"xT{c}")
            if split:  # halves so the first v-block unblocks sooner
                for hh in range(2):
                    nc.sync.dma_start_transpose(
                        out=xt[:, :, hh * 256:(hh + 1) * 256],
                        in_=xb_d.ap()[c * 512 + hh * 256:
                                      c * 512 + (hh + 1) * 256, :],
                    )
            else:
                nc.sync.dma_start_transpose(
                    out=xt, in_=xb_d.ap()[c * 512:(c + 1) * 512, :]
                )
            xTs[c] = xt

        # All DMAs go through the SP queue: the tile scheduler pins
        # cross-queue DMA order with completion semaphores (~2us dead time
        # per pin), while same-queue DMAs pipeline back-to-back.
        xt0 = xtp.tile([128, KC, 512], BF16, tag="xT", name="xT0")
        xTs[0] = xt0
        # first 256 T-rows: natural loads + PE transposes (PE is idle and
        # cold at startup; frees XBAR queue slots so wv arrives sooner).
        # idn is emitted above, BEFORE these reads (sequential semantics).
        for hh in range(2):
            xs = e_pool.tile([128, 1024], BF16, tag="e", name=f"xs{hh}")
            nc.sync.dma_start(
                out=xs, in_=xb_d.ap()[hh * 128:(hh + 1) * 128, :]
            )
            tpv = qk_ps.tile([128, 512], F32, tag="qk",
                             name=f"tpx{hh}").bitcast(BF16)
            for k in range(KC):
                nc.tensor.transpose(
                    out=tpv[:, k * 128:(k + 1) * 128],
                    in_=xs[:, k * 128:(k + 1) * 128],
                    identity=idn,
                )
            nc.vector.tensor_copy(
                out=xt0[:, :, hh * 128:(hh + 1) * 128],
                in_=tpv.rearrange("p (k t) -> p k t", t=128),
            )
        for kk in range(4):
            nc.sync.dma_start(
                out=wv_sb[:, 2 * kk:2 * kk + 2, :],
                in_=wv_d.ap()[kk * 256:(kk + 1) * 256, :].rearrange(
                    "(k p) f -> p k f", p=128),
            )
        nc.sync.dma_start_transpose(
            out=xt0[:, :, 256:512], in_=xb_d.ap()[256:512, :]
        )
        for kk in range(4):
            nc.sync.dma_start(
                out=wqk_sb[:, 2 * kk:2 * kk + 2, :],
                in_=wqk_d.ap()[kk * 256:(kk + 1) * 256, :].rearrange(
                    "(k p) f -> p k f", p=128),
            )
        nc.sync.dma_start(out=bq_sb, in_=bq_d.ap().rearrange("a p o -> p a o"))
        # xT(1)/wp are issued later (inside QKV(0)) so their DMA requests
        # cannot jump ahead of wqk on the shared DMA engines.


        # ---- work units ----
        # unit order: v blocks first (smaller weight tensor arrives first at
        # startup), then qk m-blocks interleaved q/k so that heads 2l/2l+1
        # (which read q block l and k block 4+l) unblock in head order.
        UNIT_ORDER = [("v", 0), ("v", 1), ("v", 2), ("v", 3),
                      ("m", 0), ("m", 4), ("m", 1), ("m", 5),
                      ("m", 2), ("m", 6), ("m", 3), ("m", 7)]

        def qkv_unit(c, u):
            kind, idx = UNIT_ORDER[u]
            if kind == "v":
                tt = idx
                pv = mm_ps.tile([128, CL], F32, tag="mm", name=f"pv{c}_{tt}")
                for k in range(KC):
                    nc.tensor.matmul(
                        out=pv,
                        lhsT=xTs[c][:, k, tt * 128:(tt + 1) * 128],
                        rhs=wv_sb[:, k, :],
                        start=(k == 0),
                        stop=(k == KC - 1),
                    )
                nc.vector.tensor_copy(
                    out=vaug[:, 4 * c + tt, :, 0:64],
                    in_=pv.rearrange("p (h d) -> p h d", d=64),
                )
            else:
                m = idx
                pq = mm_ps.tile([128, 512], F32, tag="mm", name=f"pq{c}_{m}")
                for k in range(KC):
                    nc.tensor.matmul(
                        out=pq,
                        lhsT=wqk_sb[:, k, m * 128:(m + 1) * 128],
                        rhs=xTs[c][:, k, :],
                        start=(k == 0),
                        stop=(k == KC - 1),
                    )
                dst = qkT[:, m, c * 512:(c + 1) * 512]
                if m < 4:  # q: add per-partition bias
                    if c <= 1:
                        nc.scalar.activation(
                            out=dst, in_=pq,
                            func=mybir.ActivationFunctionType.Identity,
                            bias=bq_sb[:, m, :], scale=1.0,
                        )
                    else:
                        nc.vector.tensor_scalar_add(dst, pq, bq_sb[:, m, :])
                elif c <= 1:
                    nc.scalar.copy(out=dst, in_=pq)
                else:
                    nc.vector.tensor_copy(out=dst, in_=pq)

        def scores_pair(c, l, pj, e_tiles):
            row = (l % 2) * 64
            qtile = l // 2
            ktile = 4 + l // 2
            j0 = 2 * pj - 4 * c
            los = {0: (0, 128), 2: (256, 384)}.get(j0, None)
            if j0 == 2:
                pqk = qkd_ps.tile([128, 384], F32, tag="qkd",
                                  name=f"pqk{c}_{l}_{pj}")
            else:
                pqk = qk_ps.tile([128, 1024], F32, tag="qk",
                                 name=f"pqk{c}_{l}_{pj}")
            e = e_pool.tile([128, 1024], BF16, tag="e", name=f"e{c}_{l}_{pj}")
            if los:
                # diagonal pairs: both halves' valid windows packed
                # adjacently -> one exp + one mask-mul for the pair
                lo0, lo1 = los
                w0, w1 = 512 - lo0, 512 - lo1
                for h, (lo, base) in enumerate(((lo0, 0), (lo1, w0))):
                    nc.tensor.matmul(
                        out=pqk[:, base:base + 512 - lo],
                        lhsT=qkT[row:row + 64, ktile,
                                 (2 * pj + h) * 128:(2 * pj + h + 1) * 128],
                        rhs=qkT[row:row + 64, qtile,
                                c * 512 + lo:(c + 1) * 512],
                        start=True,
                        stop=True,
                    )
                w = w0 + w1
                nc.scalar.activation(
                    out=e[:, 0:w], in_=pqk[:, 0:w], func=EXP, scale=SCALE,
                )
                if w == 896:
                    # only the two 128-wide diagonal triangle sub-blocks
                    # can be invalid; mask just those
                    tri128 = m384[:, 256:384]
                    nc.vector.tensor_mul(e[:, 0:128], e[:, 0:128], tri128)
                    nc.vector.tensor_mul(e[:, 512:640], e[:, 512:640], tri128)
                else:
                    nc.vector.tensor_mul(e[:, 0:w], e[:, 0:w], m384)
            else:
                for h in range(2):
                    tk = 2 * pj + h
                    nc.tensor.matmul(
                        out=pqk[:, h * 512:(h + 1) * 512],
                        lhsT=qkT[row:row + 64, ktile,
                                 tk * 128:(tk + 1) * 128],
                        rhs=qkT[row:row + 64, qtile,
                                c * 512:(c + 1) * 512],
                        start=True,
                        stop=True,
                    )
                nc.scalar.activation(out=e, in_=pqk, func=EXP, scale=SCALE)
            e_tiles.append(e)

        yv_tiles = {}

        def av_chain(c, l, qq, e_tiles):
            if l not in yv_tiles:
                yvt = mm_ps.tile([128, 512], F32, tag="mm", name=f"yv{c}_{l}")
                yv_tiles[l] = yvt[:, 0:260].rearrange("p (q e) -> p q e", e=65)
            yv = yv_tiles[l]
            nk = 4 * c + qq + 1
            for tk in range(nk):
                pj, h = tk // 2, tk % 2
                j0 = 2 * pj - 4 * c
                los = {0: (0, 128), 2: (256, 384)}.get(j0, None)
                if los:  # packed diagonal-pair e layout
                    lo0, lo1 = los
                    col = (qq * 128 - lo0) if h == 0 else \
                        (512 - lo0) + (qq * 128 - lo1)
                else:
                    col = h * 512 + qq * 128
                nc.tensor.matmul(
                    out=yv[:, qq, :],
                    lhsT=e_tiles[pj][:, col:col + 128],
                    rhs=vaug[:, tk, l, :],
                    start=(tk == 0),
                    stop=(tk == nk - 1),
                )

        def norm_head(c, l, ynat):
            yv = yv_tiles.pop(l)
            rc = rcp.tile([128, 4], F32, tag="rc", name=f"rc{c}_{l}")
            nc.vector.reciprocal(out=rc, in_=yv[:, :, 64])
            nc.vector.tensor_mul(
                ynat[:, :, l * 64:(l + 1) * 64],
                yv[:, :, 0:64],
                rc.unsqueeze(2).broadcast_to((128, 4, 64)),
            )

        os_tiles = {}

        def proj_unit(c, u):
            tq = 4 * c + u // 2
            oc = u % 2
            if oc == 0:
                os_tiles[tq] = osp.tile([128, 1024], F32, tag="os",
                                        name=f"os{tq}")
            os_ = os_tiles[tq]
            pp = mm_ps.tile([128, 512], F32, tag="mm", name=f"pp{tq}_{oc}")
            for k in range(4):
                nc.tensor.matmul(
                    out=pp,
                    lhsT=yT[:, k, tq * 128:(tq + 1) * 128],
                    rhs=wp_sb[:, k, oc * 512:(oc + 1) * 512],
                    start=(k == 0),
                    stop=(k == 3),
                )
            nc.vector.tensor_copy(out=os_[:, oc * 512:(oc + 1) * 512], in_=pp)
            nc.sync.dma_start(
                out=out_d.ap()[tq * 128:(tq + 1) * 128,
                               oc * 512:(oc + 1) * 512],
                in_=os_[:, oc * 512:(oc + 1) * 512],
            )
            if oc == 1:
                del os_tiles[tq]

        # ---- main pipeline ----
        # QKV(0) first 8 units straight (PE's first work; heads 0-3 of
        # chunk 0 unblock), rest deferred into chunk 0's filler stream.
        # Per chunk c: the scores of head l+1 are interleaved at pair
        # granularity with head l's AV chains and with QKV/proj filler
        # units, so PE never sits in an ACT-paced scores run. proj(1) and
        # proj(2) are both deferred to chunk 3, where ACT exp pressure
        # peaks and PE needs the most filler work.
        for u in range(8):
            qkv_unit(0, u)
            if u == 4:
                emit_xt(1)
            if u == 6:
                nc.sync.dma_start(
                    out=wp_sb,
                    in_=wp_d.ap().rearrange("(k p) f -> p k f", p=128),
                )

        for c in range(NT):
            npairs = 2 * c + 2
            if c + 2 < NT:
                emit_xt(c + 2)
            fillers = deque()
            if c == 0:
                for u in range(8, 12):
                    fillers.append(lambda u=u: qkv_unit(0, u))
            if c + 1 < NT:
                for u in range(12):
                    fillers.append(lambda u=u: qkv_unit(c + 1, u))
            if c == 3:
                for cc in (0, 1, 2):
                    for u in range(8):
                        fillers.append(lambda cc=cc, u=u: proj_unit(cc, u))
            nf = len(fillers)
            total_slots = HL * npairs
            done = [0]

            def tick(slot, nf=nf, total_slots=total_slots, done=done,
                     fillers=fillers):
                want = (nf * slot) // total_slots
                while done[0] < want and fillers:
                    fillers.popleft()()
                    done[0] += 1

            ynat = ynp.tile([128, 4, 512], BF16, tag="ynat", name=f"ynat{c}")
            e_heads = [[] for _ in range(HL)]
            for pj in range(npairs):
                scores_pair(c, 0, pj, e_heads[0])
            for l in range(HL):
                # spread head l's 4 AV chains across head l+1's score pairs
                if l + 1 < HL:
                    av_at = {((qq + 1) * npairs) // 5: qq for qq in range(4)}
                    for pj in range(npairs):
                        scores_pair(c, l + 1, pj, e_heads[l + 1])
                        tick(l * npairs + pj + 1)
                        if pj in av_at:
                            av_chain(c, l, av_at[pj], e_heads[l])
                    for qq in range(4):  # c=0 has only 2 pair slots
                        if qq not in av_at.values():
                            av_chain(c, l, qq, e_heads[l])
                elif c == 3:
                    # last head of the last chunk: pipeline recip/norm/PE
                    # transpose per query block right behind each AV chain
                    # so the tail's projections start as early as possible
                    yvt = None
                    tp = qk_ps.tile([128, 1024], F32, tag="qk",
                                    name="tps3").bitcast(BF16)
                    for qq in range(4):
                        av_chain(c, l, qq, e_heads[l])
                        yvt = yv_tiles[l]
                        rcq = rcp.tile([128, 1], F32, tag="rc",
                                       name=f"rcq{qq}")
                        nc.vector.reciprocal(out=rcq, in_=yvt[:, qq, 64:65])
                        nc.vector.tensor_mul(
                            ynat[:, qq, l * 64:(l + 1) * 64],
                            yvt[:, qq, 0:64],
                            rcq.broadcast_to((128, 64)),
                        )
                        nc.tensor.transpose(
                            out=tp[:, qq * 128:(qq + 1) * 128],
                            in_=ynat[:, qq, 384:512],
                            identity=idn,
                        )
                        nc.vector.tensor_copy(
                            out=yT[:, 3, (12 + qq) * 128:(13 + qq) * 128],
                            in_=tp[:, qq * 128:(qq + 1) * 128],
                        )
                    del yv_tiles[l]
                else:
                    for qq in range(4):
                        av_chain(c, l, qq, e_heads[l])
                        tick(l * npairs + ((qq + 1) * npairs) // 4)
                if not (c == 3 and l == 7):
                    norm_head(c, l, ynat)
                if c == 3 and l % 2 == 1 and l < 7:
                    p = l // 2
                    for qq in range(4):
                        nc.sync.dma_start_transpose(
                            out=yT[:, p,
                                   (4 * c + qq) * 128:(4 * c + qq + 1) * 128],
                            in_=ynat[:, qq, p * 128:(p + 1) * 128],
                        )
            while fillers:
                fillers.popleft()()
            if c < 3:
                for qq in range(4):
                    nc.sync.dma_start_transpose(
                        out=yT[:, :, (4 * c + qq) * 128:(4 * c + qq + 1) * 128],
                        in_=ynat[:, qq, :],
                    )
        for u in range(8):
            proj_unit(3, u)


# --------------------------------------------------------------------------
# Cached PJRT execution (reuses the compiled executable across calls).
# --------------------------------------------------------------------------
_CACHE = {}


def _get_runner(repeat=1):
    key = ("runner", repeat)
    if key in _CACHE:
        return _CACHE[key]

    import jax
    from jax.sharding import Mesh, PartitionSpec
    from jax.experimental.shard_map import shard_map
    from concourse import bass2jax

    nc = build_bass(repeat=repeat)
    bass2jax.install_neuronx_cc_hook()

    partition_name = (
        nc.partition_id_tensor.name if nc.partition_id_tensor else None
    )
    in_names, out_names, out_avals, zero_shapes = [], [], [], []
    for alloc in nc.m.functions[0].allocations:
        if not isinstance(alloc, mybir.MemoryLocationSet):
            continue
        name = alloc.memorylocations[0].name
        if alloc.kind == "ExternalInput":
            if name != partition_name:
                in_names.append(name)
        elif alloc.kind == "ExternalOutput":
            shape = tuple(alloc.tensor_shape)
            dtype = mybir.dt.np(alloc.dtype)
            out_names.append(name)
            out_avals.append(jax.core.ShapedArray(shape, dtype))
            zero_shapes.append((shape, dtype))
    n_params = len(in_names)
    n_outs = len(out_avals)
    all_in_names = list(in_names) + list(out_names)
    if partition_name is not None:
        all_in_names.append(partition_name)

    def _body(*args):
        operands = list(args)
        if partition_name is not None:
            operands.append(bass2jax.partition_id_tensor())
        outs = bass2jax._bass_exec_p.bind(
            *operands,
            out_avals=tuple(out_avals),
            in_names=tuple(all_in_names),
            out_names=tuple(out_names),
            lowering_input_output_aliases=(),
            sim_require_finite=True,
            sim_require_nnan=True,
            nc=nc,
        )
        return tuple(outs)

    devices = jax.devices()[:NCORES]
    mesh = Mesh(np.asarray(devices), ("core",))
    in_specs = (PartitionSpec("core"),) * (n_params + n_outs)
    out_specs = (PartitionSpec("core"),) * n_outs
    donate = tuple(range(n_params, n_params + n_outs))
    sharded = jax.jit(
        shard_map(
            _body, mesh=mesh, in_specs=in_specs, out_specs=out_specs,
            check_rep=False,
        ),
        donate_argnums=donate,
        keep_unused=True,
    )

    runner = {
        "sharded": sharded,
        "in_names": in_names,
        "out_names": out_names,
        "zero_shapes": zero_shapes,
        "n_params": n_params,
        "mesh": mesh,
    }
    _CACHE[key] = runner
    return runner


def _make_core_inputs(x, W_attn, b_attn, W_proj):
    """Per-core input dicts (core i: batch i//2, head-group i%2)."""
    import ml_dtypes

    bf16 = ml_dtypes.bfloat16
    x = np.ascontiguousarray(x, dtype=np.float32)
    W_attn = np.ascontiguousarray(W_attn, dtype=np.float32)
    b_attn = np.ascontiguousarray(b_attn, dtype=np.float32)
    W_proj = np.ascontiguousarray(W_proj, dtype=np.float32)

    per_group = []
    for g in range(2):
        s = g * CL
        wqk = np.ascontiguousarray(
            np.concatenate(
                [W_attn[:, s:s + CL], W_attn[:, C + s:C + s + CL]], axis=1
            ).astype(bf16)
        )
        wv = np.ascontiguousarray(W_attn[:, 2 * C + s:2 * C + s + CL].astype(bf16))
        bq = np.ascontiguousarray(b_attn[s:s + CL].reshape(4, 128, 1))
        wp = np.ascontiguousarray(W_proj[s:s + CL, :].astype(bf16))
        per_group.append((wqk, wv, bq, wp))

    in_maps = []
    for core in range(NCORES):
        b_i, g = core // 2, core % 2
        wqk, wv, bq, wp = per_group[g]
        in_maps.append(
            {"xb": np.ascontiguousarray(x[b_i].astype(bf16)), "wqk": wqk,
             "wv": wv, "bq": bq, "wp": wp}
        )
    return in_maps


def run_cores(in_maps, timing_reps=0, repeat=1):
    """Run the SPMD kernel. Returns (list of per-core output dicts, best_ns)."""
    import jax, time

    r = _get_runner(repeat=repeat)
    per_core = [
        [np.asarray(m[name]) for name in r["in_names"]] for m in in_maps
    ]
    concat_in = [
        np.concatenate([per_core[c][i] for c in range(NCORES)], axis=0)
        for i in range(len(r["in_names"]))
    ]

    def zeros():
        return [
            np.zeros((NCORES * s[0], *s[1:]), dt) for (s, dt) in r["zero_shapes"]
        ]

    out_arrs = r["sharded"](*concat_in, *zeros())
    outs_np = [np.asarray(a) for a in out_arrs]

    best_ns = None
    if timing_reps > 0:
        from jax.sharding import NamedSharding, PartitionSpec

        shard = NamedSharding(r["mesh"], PartitionSpec("core"))
        dev_in = [jax.device_put(a, shard) for a in concat_in]
        for a in dev_in:
            a.block_until_ready()
        zsets = []
        for _ in range(timing_reps + 1):
            zs = [jax.device_put(z, shard) for z in zeros()]
            for a in zs:
                a.block_until_ready()
            zsets.append(zs)
        res = r["sharded"](*dev_in, *zsets[0])  # warm
        for a in res:
            a.block_until_ready()
        times = []
        for i in range(timing_reps):
            t0 = time.perf_counter()
            res = r["sharded"](*dev_in, *zsets[i + 1])
            for a in res:
                a.block_until_ready()
            t1 = time.perf_counter()
            times.append(t1 - t0)
        best_ns = int(min(times) * 1e9)

    results = []
    for c in range(NCORES):
        m = {}
        for i, name in enumerate(r["out_names"]):
            full = outs_np[i]
            shape = r["zero_shapes"][i][0]
            m[name] = full.reshape(NCORES, *shape)[c]
        results.append(m)
    return results, best_ns


def kernel(x, W_attn, b_attn, W_proj, b_proj, _timing_reps=0, _return_ns=False):
    x = np.asarray(x, dtype=np.float32)
    W_attn = np.asarray(W_attn, dtype=np.float32)
    b_attn = np.asarray(b_attn, dtype=np.float32)
    W_proj = np.asarray(W_proj, dtype=np.float32)
    b_proj = np.asarray(b_proj, dtype=np.float32)

    in_maps = _make_core_inputs(x, W_attn, b_attn, W_proj)
    results, best_ns = run_cores(in_maps, timing_reps=_timing_reps)

    # v-bias contributes a constant row through the projection
    bias_row = (b_proj + b_attn[2 * C:3 * C] @ W_proj).astype(np.float32)

    out = np.empty((B, T, C), dtype=np.float32)
    for b_i in range(B):
        out[b_i] = results[2 * b_i]["out"] + results[2 * b_i + 1]["out"]
        out[b_i] += bias_row[None, :]
    if _return_ns:
        return out, best_ns
    return out
